# revision 1
# baseline (speedup 1.0000x reference)
"""CAPE connectivity loss on 8 Trainium2 NeuronCores.

Math (reference): fg_prob = softmax(logits, ch_axis)[:, 1] = sigmoid(l1 - l0);
per batch, heat diffuses from 32 source voxels for 10 iterations of
  h = avg_pool3d_3x3x3(h) * prob;  h /= (max(h) + 1e-5)
then scores = h[endpoints_b], loss = mean over batches of (1 - mean(scores)).

Key structure exploited:
 1. The 3x3x3 box filter dilates support by exactly 1 voxel per iteration, so
    after 10 iterations heat is identically zero outside L-inf radius-10 balls
    around the 32 sources. Compute only on per-cluster regions (bbox+10),
    merged until the expanded regions are pairwise disjoint - then zero-BC
    diffusion per region is exact.
 2. The per-iteration max-normalization commutes with the linear
    pool*prob step: iterate u_{k+1} = pool(u_k)*prob unnormalized, record
    mu_k = max(u_k); then h_k = u_k / c_k with c_k = mu_k + SMOOTH*c_{k-1}.
    Zero-BC values outside a piece's owned claim only *underestimate* the true
    field, and the argmax voxel lies inside some owned claim, so the max over
    all region boxes equals the true global max.
 3. Oversized regions are split along an axis with a +10-voxel halo per
    internal cut; each piece's owned claim stays exact for all 10 iterations.

Device layout: pieces are rotated (largest axis -> partitions), their source
bboxes centered, and packed into columns of [128, j, bh, bw] bf16 SBUF tiles
in two width classes, with zero guard bands around each box (2 cols left so
w-windows stay >= 2). prob = sigmoid(l1-l0) is computed on the host for the
packed boxes only and shipped directly. Per iteration on a band that tracks
the dilated support (even-aligned w-windows for DVE 2x mode), the separable
3x3x3 pool splits engine-adaptively: the D-axis 3-tap is a PE matmul against
a block-tridiagonal 1/27 matrix; the W-axis 3-tap always rides on PE as
w-shifted accumulating matmuls; the H-axis 3-tap runs as 0/1/2 DVE row-shift
adds with the remainder folded into extra h-shifted matmuls (3/6/9 matmuls
per chunk) - chosen per (class, iteration) by a cost model balancing DVE vs
PE load. The *prob multiply runs on DVE, reading PSUM directly for small
bands or via a ScalarE PSUM->SBUF copy (idle engine) for large ones. u is
updated in place; band slabs of u_8/u_9/u_10 are DMAed out and the host
computes the global maxima (scale recurrence c_10 = mu_10 + eps*mu_9 +
eps^2*mu_8 + O(eps^3)) and gathers endpoint_b values. A burst of dummy
matmuls at kernel start warms the PE HAM clock gate during the input DMA.
"""

import ml_dtypes
import numpy as np

B, C, D, H, W = 2, 2, 128, 256, 256
N_PAIRS = 32
N_ITERS = 10
SMOOTH = 1e-5
R = N_ITERS  # diffusion reach in voxels
VOL = (D, H, W)

AXIS_CAP = 34   # max free-axis extent of a piece (split with halo beyond)
P_CAP = 128     # max partition-axis extent
A_FREE = 21     # class-A pieces have both free extents <= A_FREE
N_CORES = 8
N_WARM = 64     # PE warmup matmuls (HAM un-throttle needs ~3.4us busy)
HOP_MIN = 256   # min band area for the ScalarE PSUM->SBUF hop

# Results of the last device run (for test harness introspection).
LAST_RESULTS = None


# --------------------------------------------------------------------------
# planning: clusters -> regions -> pieces
# --------------------------------------------------------------------------

class Piece:
    __slots__ = ("lo", "hi", "clo", "chi", "srcs", "batch",
                 "perm", "cls", "core", "col", "p0", "offh", "offw")

    def __init__(self, lo, hi, clo, chi, srcs, batch):
        self.lo = np.asarray(lo); self.hi = np.asarray(hi)
        self.clo = np.asarray(clo); self.chi = np.asarray(chi)
        self.srcs = srcs
        self.batch = batch

    @property
    def ext(self):
        return self.hi - self.lo + 1


def _merge_clusters(pts):
    """Merge clusters until expanded regions (bbox+R) are pairwise disjoint."""
    clusters = [[i] for i in range(len(pts))]

    def bbox(c):
        p = pts[c]
        return p.min(0), p.max(0)

    changed = True
    while changed:
        changed = False
        out = []
        while clusters:
            c = clusters.pop()
            lo_c, hi_c = bbox(c)
            for k, o in enumerate(clusters):
                lo_o, hi_o = bbox(o)
                if np.all(lo_c - hi_o <= 2 * R) and np.all(lo_o - hi_c <= 2 * R):
                    clusters[k] = o + c
                    changed = True
                    break
            else:
                out.append(c)
        clusters = out
    return clusters


def _split(piece):
    """Split a piece until partition extent <= P_CAP and free extents <=
    AXIS_CAP. Claims are halved; each internal cut adds R halo."""
    out, done = [piece], []
    while out:
        p = out.pop()
        ext = p.ext
        order = np.argsort(-ext, kind="stable")
        ax = None
        if ext[order[0]] > P_CAP:
            ax = order[0]
        elif ext[order[1]] > AXIS_CAP:
            ax = order[1]
        elif ext[order[2]] > AXIS_CAP:
            ax = order[2]
        if ax is None:
            done.append(p)
            continue
        mid = (p.clo[ax] + p.chi[ax]) // 2
        for a, b in ((p.clo[ax], mid), (mid + 1, p.chi[ax])):
            nclo, nchi = p.clo.copy(), p.chi.copy()
            nclo[ax], nchi[ax] = a, b
            nlo, nhi = p.lo.copy(), p.hi.copy()
            nlo[ax] = max(a - R, p.lo[ax])
            nhi[ax] = min(b + R, p.hi[ax])
            srcs = [s for s in p.srcs if nlo[ax] <= s[ax] <= nhi[ax]]
            out.append(Piece(nlo, nhi, nclo, nchi, srcs, p.batch))
    return done


def make_pieces(ea):
    pieces = []
    for b in range(ea.shape[0]):
        pts = np.unique(ea[b], axis=0)
        for cl in _merge_clusters(pts):
            p = pts[cl]
            lo = np.maximum(p.min(0) - R, 0)
            hi = np.minimum(p.max(0) + R, np.asarray(VOL) - 1)
            base = Piece(lo, hi, lo, hi, [tuple(x) for x in p], b)
            pieces.extend(_split(base))
    return pieces


# --------------------------------------------------------------------------
# packing: pieces -> (class, core, col, partition offset) + schedule
# --------------------------------------------------------------------------

class Cfg:
    pass


def _chunks(rows, wn):
    nch = max(1, -(-(rows * wn) // 512))
    rpc = -(-rows // nch)
    while rpc * wn > 512:
        nch += 1
        rpc = -(-rows // nch)
    return nch, rpc


def pack(pieces):
    """Assign each piece a rotation + (class, core, col, p0); build the
    per-(class, iteration) band slabs, chunking, and engine schedule."""
    for p in pieces:
        ext = p.ext
        p.perm = tuple(int(i) for i in np.argsort(-ext, kind="stable"))

    def free_ext(p):
        e = p.ext
        return e[p.perm[1]], e[p.perm[2]]

    cls_of = {}
    for p in pieces:
        fh, fw = free_ext(p)
        p.cls = 0 if (fh <= A_FREE and fw <= A_FREE) else 1
        cls_of.setdefault(p.cls, []).append(p)

    cfg = Cfg()
    cfg.classes = []
    for cls in (0, 1):
        plist = cls_of.get(cls, [])
        if not plist:
            continue
        ih = max(int(free_ext(p)[0]) for p in plist)
        iw = max(int(free_ext(p)[1]) for p in plist)
        bh = ih + 2
        bw = iw + 4 + ((iw + 4) % 2)  # 2 guard cols left, >=2 right, even
        # first-fit decreasing bin pack by partition extent
        plist.sort(key=lambda p: -int(p.ext[p.perm[0]]))
        bins = []  # (used, [pieces])
        for p in plist:
            pe = int(p.ext[p.perm[0]])
            for b_ in bins:
                if b_[0] + pe <= 128:
                    p.p0 = b_[0]
                    b_[0] += pe
                    b_[1].append(p)
                    break
            else:
                p.p0 = 0
                bins.append([pe, [p]])
        cfg.classes.append(dict(cls=cls, ih=ih, iw=iw, bh=bh, bw=bw,
                                cols=bins))

    for c in cfg.classes:
        for i, b_ in enumerate(c["cols"]):
            for p in b_[1]:
                p.core = i % N_CORES
                p.col = i // N_CORES
        c["j"] = (len(c["cols"]) + N_CORES - 1) // N_CORES

    # Center each piece's source bbox in its box interior (rows start at 1,
    # cols at 2 so shifted matmul reads never cross the box start), then
    # per-class per-iteration band slabs (h0,h1,w0,w1) with the w-window
    # even-aligned for DVE 2x mode.
    for c in cfg.classes:
        ih, iw = c["ih"], c["iw"]
        ulo = [10**9, 10**9]
        uhi = [-1, -1]
        for b_ in c["cols"]:
            for p in b_[1]:
                offs = []
                for axi, ilen, base in ((1, ih, 1), (2, iw, 2)):
                    ax = p.perm[axi]
                    ext = int(p.ext[ax])
                    if p.srcs:
                        slo = min(s[ax] for s in p.srcs) - int(p.lo[ax])
                        shi = max(s[ax] for s in p.srcs) - int(p.lo[ax])
                    else:
                        slo = shi = ext // 2
                    start = base + (ilen - 1) // 2 - (slo + shi) // 2
                    start = min(max(start, base), base + ilen - ext)
                    offs.append(start)
                    if p.srcs:
                        i01 = 0 if axi == 1 else 1
                        ulo[i01] = min(ulo[i01], start + slo)
                        uhi[i01] = max(uhi[i01], start + shi)
                p.offh, p.offw = offs
        if uhi[0] < 0:  # no sources in this class at all
            ulo, uhi = [1, 2], [ih, iw + 1]
        c["ext_k"] = []
        for k in range(1, N_ITERS + 1):
            h0 = max(1, ulo[0] - k)
            h1 = min(1 + ih, uhi[0] + k + 1)
            w0 = max(2, ulo[1] - k)
            w1 = min(2 + iw, uhi[1] + k + 1)
            w0 -= w0 % 2
            wn = w1 - w0
            wn += wn % 2
            c["ext_k"].append((int(h0), int(h1), int(w0), int(w0 + wn)))

    # Engine schedule. The PE is power-throttled to ~50% duty under
    # sustained load (~1.2 ns/col effective), so DVE 2x (0.53 ns/elem) is
    # the fastest engine per element: the W-axis 3-tap always runs as 2 DVE
    # shift-adds; per (iteration, class) the H-axis 3-tap runs as n_t in
    # {2, 1, 0} further DVE ops (2 = full H on DVE + 1 matmul; 1 = H pair
    # on DVE + 2 matmuls; 0 = H folded into 3 h-shifted matmuls), greedily
    # balancing cumulative DVE vs PE busy-time. The psum->u multiply hops
    # through a chunk-pipelined ScalarE copy (idle engine) for bands >=
    # HOP_MIN so the DVE multiply runs from SBUF at 2x.
    dve = pe = 0.0
    for it in range(N_ITERS):
        for c in cfg.classes:
            j = c["j"]
            h0, h1, w0, w1 = c["ext_k"][it]
            rows, wn = h1 - h0, w1 - w0
            A = rows * wn * j
            nch, rpc = _chunks(rows, wn)
            ach = rpc * wn
            hop = A >= HOP_MIN
            mcost = (nch * (170 + 0.53 * ach) if hop
                     else nch * (192 + 1.25 * ach))
            best = None
            for n in (2, 1, 0):
                d = (2 + n) * (170 + 0.53 * A) + mcost * j
                pcost = nch * (3 - n) * (ach * 1.2 + 160) * j
                tot = max(dve + d, pe + pcost)
                if best is None or tot < best[0]:
                    best = (tot, n, d, pcost)
            _, n, d, pcost = best
            dve += d
            pe += pcost
            c.setdefault("sched", []).append(
                dict(n_t=n, hop=hop, nch=nch, rpc=rpc))
    cfg.pieces = pieces
    return cfg


# --------------------------------------------------------------------------
# host-side data packing
# --------------------------------------------------------------------------

def build_inputs(cfg, logits, ea):
    """Build per-core input arrays. Returns in_maps (list of dicts)."""
    bf16 = ml_dtypes.bfloat16
    in_maps = [dict() for _ in range(N_CORES)]
    for ci, c in enumerate(cfg.classes):
        j, bh, bw = c["j"], c["bh"], c["bw"]
        for core in range(N_CORES):
            # prob defaults to 0 outside regions: no heat leaks through box
            # cells beyond a (clipped) region slab, matching the true zero BC.
            in_maps[core][f"pr_{ci}"] = np.zeros((128, j, bh, bw), bf16)
            in_maps[core][f"u0_{ci}"] = np.zeros((128, j, bh, bw), bf16)
            in_maps[core][f"tm_{ci}"] = np.zeros((128, j, 128), bf16)

    inv27 = np.float32(1.0 / 27.0)
    for p in cfg.pieces:
        ci = next(i for i, c in enumerate(cfg.classes) if c["cls"] == p.cls)
        lo, hi, perm = p.lo, p.hi, p.perm
        pe = int(p.ext[perm[0]])
        eh = int(p.ext[perm[1]])
        ew = int(p.ext[perm[2]])
        sl = tuple(slice(int(lo[a]), int(hi[a]) + 1) for a in range(3))
        oh, ow = p.offh, p.offw
        dlt = (logits[p.batch, 1][sl].astype(np.float32)
               - logits[p.batch, 0][sl].astype(np.float32)).transpose(perm)
        prob = 1.0 / (1.0 + np.exp(-dlt))
        in_maps[p.core][f"pr_{ci}"][p.p0:p.p0 + pe, p.col,
                                    oh:oh + eh, ow:ow + ew] = \
            prob.astype(bf16)
        u0 = in_maps[p.core][f"u0_{ci}"]
        for s in p.srcs:
            q = (s[perm[0]] - lo[perm[0]], s[perm[1]] - lo[perm[1]],
                 s[perm[2]] - lo[perm[2]])
            u0[p.p0 + q[0], p.col, oh + q[1], ow + q[2]] = 1.0
        tm = in_maps[p.core][f"tm_{ci}"]
        for i in range(pe):
            for d_ in (-1, 0, 1):
                if 0 <= i + d_ < pe:
                    tm[p.p0 + i, p.col, p.p0 + i + d_] = inv27
    return in_maps


# --------------------------------------------------------------------------
# device kernel
# --------------------------------------------------------------------------

def build_nc(cfg):
    import concourse.bacc as bacc
    import concourse.tile as tile
    from concourse import mybir

    nc = bacc.Bacc("TRN2")
    dram = {}
    for ci, c in enumerate(cfg.classes):
        j, bh, bw = c["j"], c["bh"], c["bw"]
        for nm in ("pr", "u0"):
            dram[f"{nm}_{ci}"] = nc.dram_tensor(
                f"{nm}_{ci}", [128, j, bh, bw], mybir.dt.bfloat16,
                kind="ExternalInput")
        dram[f"tm_{ci}"] = nc.dram_tensor(
            f"tm_{ci}", [128, j, 128], mybir.dt.bfloat16, kind="ExternalInput")
        # full-width row ranges: per-partition contiguous runs so the DMA
        # moves ~2KB packets instead of one tiny packet per row
        for oi in (8, 9, 10):
            h0, h1, w0, w1 = c["ext_k"][oi - 1]
            dram[f"o{oi}_{ci}"] = nc.dram_tensor(
                f"o{oi}_{ci}", [128, j, h1 - h0, bw],
                mybir.dt.bfloat16, kind="ExternalOutput")

    with tile.TileContext(nc) as tc:
        with tc.tile_pool(name="sb", bufs=1) as sb, \
             tc.tile_pool(name="ps", bufs=7, space="PSUM") as pp:
            tiles = []
            for ci, c in enumerate(cfg.classes):
                j, bh, bw = c["j"], c["bh"], c["bw"]
                u = sb.tile([128, j, bh, bw], mybir.dt.bfloat16, tag=f"u{ci}")
                t1 = sb.tile([128, j, bh, bw], mybir.dt.bfloat16,
                             tag=f"t1{ci}")
                t2 = sb.tile([128, j, bh, bw], mybir.dt.bfloat16,
                             tag=f"t2{ci}")
                pr = sb.tile([128, j, bh, bw], mybir.dt.bfloat16,
                             tag=f"pr{ci}")
                tm = sb.tile([128, j, 128], mybir.dt.bfloat16, tag=f"tm{ci}")
                tiles.append((u, t1, t2, pr, tm))
            # inputs spread over three trigger queues (parallel DMA
            # engines); u0_0 gates the first taps, so its transfer is
            # split across two queues to halve the per-packet serial cost
            u0t = tiles[0][0]
            nc.sync.dma_start(out=u0t[0:64], in_=dram["u0_0"][0:64])
            nc.gpsimd.dma_start(out=u0t[64:128], in_=dram["u0_0"][64:128])
            for ci, c in enumerate(cfg.classes):
                u, t1, t2, pr, tm = tiles[ci]
                qa = nc.sync if ci == 0 else nc.gpsimd
                if ci > 0:
                    qa.dma_start(out=u[:], in_=dram[f"u0_{ci}"][:])
                qa.dma_start(out=tm[:], in_=dram[f"tm_{ci}"][:])
                nc.scalar.dma_start(out=pr[:], in_=dram[f"pr_{ci}"][:])

            def emit_taps(ci, it):
                # W-axis 3-tap on DVE (2 shift-adds), written 1 row wider
                # than the band so later H-tap/matmul reads touch only
                # written cells (u's ring is zero from the u0 DMA - no
                # memsets needed). Then the H-axis 3-tap as n_t in {2,1,0}
                # more DVE ops (the rest rides on PE as shifted matmuls).
                c = cfg.classes[ci]
                u, t1, t2, pr, tm = tiles[ci]
                h0, h1, w0, w1 = c["ext_k"][it]
                n_t = c["sched"][it]["n_t"]
                g0, g1 = h0 - 1, h1 + 1
                nc.vector.tensor_add(t1[:, :, g0:g1, w0:w1],
                                     u[:, :, g0:g1, w0 - 1:w1 - 1],
                                     u[:, :, g0:g1, w0 + 1:w1 + 1])
                nc.vector.tensor_add(t1[:, :, g0:g1, w0:w1],
                                     t1[:, :, g0:g1, w0:w1],
                                     u[:, :, g0:g1, w0:w1])
                if n_t >= 1:
                    nc.vector.tensor_add(t2[:, :, h0:h1, w0:w1],
                                         t1[:, :, h0 - 1:h1 - 1, w0:w1],
                                         t1[:, :, h0 + 1:h1 + 1, w0:w1])
                if n_t == 2:
                    nc.vector.tensor_add(t2[:, :, h0:h1, w0:w1],
                                         t2[:, :, h0:h1, w0:w1],
                                         t1[:, :, h0:h1, w0:w1])

            def emit_mms(ci, it):
                # D-axis tridiagonal matmul per chunk; residual H-axis taps
                # ride along as h-shifted accumulating matmuls. Then each
                # psum chunk hops through ScalarE (PSUM->SBUF bf16, idle
                # engine, overwriting t2) for bands >= HOP_MIN.
                c = cfg.classes[ci]
                u, t1, t2, pr, tm = tiles[ci]
                h0, h1, w0, w1 = c["ext_k"][it]
                sch = c["sched"][it]
                n_t, nch, rpc = sch["n_t"], sch["nch"], sch["rpc"]
                if n_t == 2:       # t2 = full H 3-tap sum
                    srcs = [(t2, 0)]
                elif n_t == 1:     # t2 = t1(h-1)+t1(h+1), center = t1
                    srcs = [(t2, 0), (t1, 0)]
                else:              # full H 3-tap as shifted matmuls
                    srcs = [(t1, -1), (t1, 0), (t1, 1)]
                psums = []
                for jj in range(c["j"]):
                    for ch in range(nch):
                        r0 = h0 + ch * rpc
                        nr = min(rpc, h1 - r0)
                        ps = pp.tile([128, nr, w1 - w0],
                                     mybir.dt.float32, tag="ps")
                        for mi, (src, dh) in enumerate(srcs):
                            nc.tensor.matmul(
                                ps[:],
                                tm[:, jj, :],
                                src[:, jj, r0 + dh:r0 + dh + nr, w0:w1],
                                start=(mi == 0),
                                stop=(mi == len(srcs) - 1))
                        psums.append((jj, r0, nr, ps))
                if sch["hop"]:
                    # chunk 0 skips the hop (its multiply reads PSUM
                    # directly) - it sits first on the critical path and
                    # saving the ScalarE round-trip starts it ~0.6us sooner
                    for ki, (jj, r0, nr, ps) in enumerate(psums):
                        if ki == 0 and len(psums) >= 2:
                            continue
                        nc.scalar.activation(
                            t2[:, jj, r0:r0 + nr, w0:w1], ps[:],
                            mybir.ActivationFunctionType.Copy)
                return psums

            def emit_mults(ci, it, psums):
                # u = psum * prob (1/27 folded into tm), chunk-level so
                # early chunks run while later chunks are still matmuling.
                c = cfg.classes[ci]
                u, t1, t2, pr, tm = tiles[ci]
                h0, h1, w0, w1 = c["ext_k"][it]
                hop = c["sched"][it]["hop"]
                for ki, (jj, r0, nr, ps) in enumerate(psums):
                    direct = (not hop) or (ki == 0 and len(psums) >= 2)
                    src = ps[:] if direct else t2[:, jj, r0:r0 + nr, w0:w1]
                    nc.vector.tensor_mul(u[:, jj, r0:r0 + nr, w0:w1],
                                         src, pr[:, jj, r0:r0 + nr, w0:w1])
                if it >= 7:
                    # gpsimd queue: idle mid-kernel, so the slab transfer
                    # starts immediately and the WAR on u clears sooner
                    nc.gpsimd.dma_start(out=dram[f"o{it + 1}_{ci}"][:],
                                        in_=u[:, :, h0:h1, :])

            # Software-pipelined schedule: class 1's psum->u multiplies are
            # deferred into the next iteration so the in-order DVE queue
            # always has ready work (class 0's taps/multiply) while class
            # 1's matmul+copy chain drains. The Tile scheduler would undo
            # this (its cost model assumes an unthrottled PE and hoists the
            # stalling multiplies), so each phase is pinned with a
            # monotonically increasing bass_wait_until_ts pseudo-time.
            def tw(ns):
                tc.tile_set_cur_wait(ns * 1e-6)

            def dur(ci, it, what):
                c = cfg.classes[ci]
                h0, h1, w0, w1 = c["ext_k"][it]
                A = (h1 - h0) * (w1 - w0) * c["j"]
                sch = c["sched"][it]
                if what == "taps":
                    return (2 + sch["n_t"]) * (190 + 0.55 * A)
                if what == "mult":
                    per = (170 + 0.53 * A / sch["nch"]) if sch["hop"] \
                        else (192 + 1.25 * A / sch["nch"])
                    return sch["nch"] * per
                ach = A / sch["nch"]
                return sch["nch"] * (3 - sch["n_t"]) * (ach * 1.2 + 160)

            if len(cfg.classes) == 1:
                for it in range(N_ITERS):
                    emit_taps(0, it)
                    emit_mults(0, it, emit_mms(0, it))
            else:
                pend = None
                t = 3000.0
                for it in range(N_ITERS - 1):
                    tw(t)
                    emit_taps(0, it)
                    tb = t + dur(0, it, "taps")
                    tw(tb)
                    ps0 = emit_mms(0, it)
                    if pend is not None:
                        tw(tb + 200)
                        emit_mults(1, it - 1, pend)
                        tb += 200 + dur(1, it - 1, "mult")
                    tw(tb + 100)
                    emit_mults(0, it, ps0)
                    tb += 100 + dur(0, it, "mult")
                    tw(tb)
                    emit_taps(1, it)
                    tb += dur(1, it, "taps")
                    tw(tb)
                    pend = emit_mms(1, it)
                    t = tb + 400
                # Final iteration with roles swapped: class 1 (the long
                # matmul+copy chain) is issued first and class 0's short
                # chain drains the tail.
                it = N_ITERS - 1
                tw(t)
                emit_mults(1, it - 1, pend)
                t += dur(1, it - 1, "mult")
                tw(t)
                emit_taps(1, it)
                t += dur(1, it, "taps")
                tw(t)
                ps1 = emit_mms(1, it)
                tw(t + 200)
                emit_taps(0, it)
                t += 200 + dur(0, it, "taps")
                tw(t)
                ps0 = emit_mms(0, it)
                tw(t + 1400)
                emit_mults(1, it, ps1)
                tw(t + 1400 + dur(1, it, "mult"))
                emit_mults(0, it, ps0)
    nc.compile()
    return nc


# --------------------------------------------------------------------------
# host-side finalization
# --------------------------------------------------------------------------

def finalize(cfg, results, eb):
    """results: list of per-core dicts with o8/o9/o10 band slabs."""
    cls_idx = {c["cls"]: i for i, c in enumerate(cfg.classes)}

    # Global maxima of u_8/u_9/u_10 from the slabs; c_10 = mu_10 + eps*mu_9
    # + eps^2*mu_8 + O(eps^3) with eps = SMOOTH = 1e-5 (~1e-15 truncation).
    mus = np.zeros((B, 3), dtype=np.float64)
    for p in cfg.pieces:
        ci = cls_idx[p.cls]
        pe = int(p.ext[p.perm[0]])
        for oi in (8, 9, 10):
            m = results[p.core][f"o{oi}_{ci}"][p.p0:p.p0 + pe, p.col]
            mus[p.batch, oi - 8] = max(mus[p.batch, oi - 8],
                                       float(m.max()))

    per_batch = []
    for b in range(B):
        cscale = 1.0
        for it in range(3):
            if mus[b, it] > 0:
                cscale = mus[b, it] + SMOOTH * cscale
        scores = []
        for e in eb[b]:
            val = 0.0
            for p in cfg.pieces:
                if p.batch != b:
                    continue
                if np.all(p.clo <= e) and np.all(e <= p.chi):
                    ci = cls_idx[p.cls]
                    c = cfg.classes[ci]
                    h0, h1, _, _ = c["ext_k"][N_ITERS - 1]
                    q = (int(e[p.perm[0]] - p.lo[p.perm[0]]),
                         int(e[p.perm[1]] - p.lo[p.perm[1]]),
                         int(e[p.perm[2]] - p.lo[p.perm[2]]))
                    hs = p.offh + q[1] - h0
                    ws = p.offw + q[2]
                    if 0 <= hs < h1 - h0:
                        val = float(results[p.core][f"o10_{ci}"]
                                    [p.p0 + q[0], p.col, hs, ws])
                    break
            scores.append(val / cscale)
        per_batch.append(1.0 - np.float32(np.mean(np.asarray(scores,
                                                             np.float32))))
    return np.array(np.mean(np.asarray(per_batch, np.float32)),
                    dtype=np.float32)


# --------------------------------------------------------------------------
# entry point
# --------------------------------------------------------------------------

def kernel(logits, labels, endpoints_a, endpoints_b):
    global LAST_RESULTS
    logits = np.asarray(logits)
    ea = np.asarray(endpoints_a).astype(np.int64)
    eb = np.asarray(endpoints_b).astype(np.int64)

    cfg = pack(make_pieces(ea))
    in_maps = build_inputs(cfg, logits, ea)
    nc = build_nc(cfg)

    from concourse.bass_utils import run_bass_kernel_spmd
    res = run_bass_kernel_spmd(nc, in_maps, core_ids=list(range(N_CORES)))
    LAST_RESULTS = res
    return finalize(cfg, res.results, eb)


if __name__ == "__main__":
    ins = {k: np.load(f"/tmp/in_{k}.npy")
           for k in ("logits", "labels", "endpoints_a", "endpoints_b")}
    out = kernel(**ins)
    print("kernel loss:", repr(out))



# revision 3
# speedup vs baseline: 1.0036x; 1.0036x over previous
"""CAPE connectivity loss on 8 Trainium2 NeuronCores — v2.

Same region-decomposition math as v1 (see kernel.py docstring): per-cluster
zero-BC diffusion on packed [128, j, bh, bw] bf16 boxes, unnormalized
iterates u_k with the scale recurrence folded on host from mu_8..mu_10.

v2 engine plan (from trace-calibrated cost model + event simulation):
 - u0 is built ON DEVICE: Pool iota writes flat indices into per-class fp32
   tiles; a DVE tensor_scalar is_equal against a per-partition target column
   (tiny DMA) stamps the one-hot sources. Kills the 503KB u0 DMA and its
   latency.
 - Four engines share the per-iteration pipeline: W/H taps and multiplies
   are assigned per-op to DVE (0.52 ns/col bf16) or Pool (1.75 ns/col) by a
   greedy finish-time scheduler; the D-axis tridiagonal matmul (+ H taps
   folded as shifted matmuls when PE has slack) runs on PE; PSUM->SBUF hops
   on the Scalar engine.
 - n_t=1 iterations order the center (t1) matmul FIRST so it overlaps the
   H1 tap; the t2 matmul accumulates after.
 - Class-1 multiplies are deferred one iteration (software pipeline) as in
   v1; emission is globally sorted by simulated start time and pinned with
   monotone tile_set_cur_wait pseudo-times.
 - Outputs (u_8/9/10 band slabs) ride the SP (sync) queue, which is
   otherwise idle after the input triggers.
"""

import ml_dtypes
import numpy as np

B, C, D, H, W = 2, 2, 128, 256, 256
N_PAIRS = 32
N_ITERS = 10
SMOOTH = 1e-5
R = N_ITERS
VOL = (D, H, W)

AXIS_CAP = 34
P_CAP = 128
A_FREE = 21
N_CORES = 8

LAST_RESULTS = None


# --------------------------------------------------------------------------
# planning: clusters -> regions -> pieces  (unchanged from v1)
# --------------------------------------------------------------------------

class Piece:
    __slots__ = ("lo", "hi", "clo", "chi", "srcs", "batch",
                 "perm", "cls", "core", "col", "p0", "offh", "offw")

    def __init__(self, lo, hi, clo, chi, srcs, batch):
        self.lo = np.asarray(lo); self.hi = np.asarray(hi)
        self.clo = np.asarray(clo); self.chi = np.asarray(chi)
        self.srcs = srcs
        self.batch = batch

    @property
    def ext(self):
        return self.hi - self.lo + 1


def _merge_clusters(pts):
    clusters = [[i] for i in range(len(pts))]

    def bbox(c):
        p = pts[c]
        return p.min(0), p.max(0)

    changed = True
    while changed:
        changed = False
        out = []
        while clusters:
            c = clusters.pop()
            lo_c, hi_c = bbox(c)
            for k, o in enumerate(clusters):
                lo_o, hi_o = bbox(o)
                if np.all(lo_c - hi_o <= 2 * R) and np.all(lo_o - hi_c <= 2 * R):
                    clusters[k] = o + c
                    changed = True
                    break
            else:
                out.append(c)
        clusters = out
    return clusters


def _split(piece):
    out, done = [piece], []
    while out:
        p = out.pop()
        ext = p.ext
        order = np.argsort(-ext, kind="stable")
        ax = None
        if ext[order[0]] > P_CAP:
            ax = order[0]
        elif ext[order[1]] > AXIS_CAP:
            ax = order[1]
        elif ext[order[2]] > AXIS_CAP:
            ax = order[2]
        if ax is None:
            done.append(p)
            continue
        mid = (p.clo[ax] + p.chi[ax]) // 2
        for a, b in ((p.clo[ax], mid), (mid + 1, p.chi[ax])):
            nclo, nchi = p.clo.copy(), p.chi.copy()
            nclo[ax], nchi[ax] = a, b
            nlo, nhi = p.lo.copy(), p.hi.copy()
            nlo[ax] = max(a - R, p.lo[ax])
            nhi[ax] = min(b + R, p.hi[ax])
            srcs = [s for s in p.srcs if nlo[ax] <= s[ax] <= nhi[ax]]
            out.append(Piece(nlo, nhi, nclo, nchi, srcs, p.batch))
    return done


def make_pieces(ea):
    pieces = []
    for b in range(ea.shape[0]):
        pts = np.unique(ea[b], axis=0)
        for cl in _merge_clusters(pts):
            p = pts[cl]
            lo = np.maximum(p.min(0) - R, 0)
            hi = np.minimum(p.max(0) + R, np.asarray(VOL) - 1)
            base = Piece(lo, hi, lo, hi, [tuple(x) for x in p], b)
            pieces.extend(_split(base))
    return pieces


# --------------------------------------------------------------------------
# packing (geometry identical to v1)
# --------------------------------------------------------------------------

class Cfg:
    pass


def _chunks(rows, wn):
    nch = max(1, -(-(rows * wn) // 512))
    rpc = -(-rows // nch)
    while rpc * wn > 512:
        nch += 1
        rpc = -(-rows // nch)
    return nch, rpc


def pack(pieces):
    for p in pieces:
        ext = p.ext
        p.perm = tuple(int(i) for i in np.argsort(-ext, kind="stable"))

    def free_ext(p):
        e = p.ext
        return e[p.perm[1]], e[p.perm[2]]

    cls_of = {}
    for p in pieces:
        fh, fw = free_ext(p)
        p.cls = 0 if (fh <= A_FREE and fw <= A_FREE) else 1
        cls_of.setdefault(p.cls, []).append(p)

    cfg = Cfg()
    cfg.classes = []
    for cls in (0, 1):
        plist = cls_of.get(cls, [])
        if not plist:
            continue
        ih = max(int(free_ext(p)[0]) for p in plist)
        iw = max(int(free_ext(p)[1]) for p in plist)
        bh = ih + 2
        bw = iw + 4 + ((iw + 4) % 2)
        plist.sort(key=lambda p: -int(p.ext[p.perm[0]]))
        bins = []
        for p in plist:
            pe = int(p.ext[p.perm[0]])
            for b_ in bins:
                if b_[0] + pe <= 128:
                    p.p0 = b_[0]
                    b_[0] += pe
                    b_[1].append(p)
                    break
            else:
                p.p0 = 0
                bins.append([pe, [p]])
        cfg.classes.append(dict(cls=cls, ih=ih, iw=iw, bh=bh, bw=bw,
                                cols=bins))

    for c in cfg.classes:
        for i, b_ in enumerate(c["cols"]):
            for p in b_[1]:
                p.core = i % N_CORES
                p.col = i // N_CORES
        c["j"] = (len(c["cols"]) + N_CORES - 1) // N_CORES

    for c in cfg.classes:
        ih, iw = c["ih"], c["iw"]
        ulo = [10**9, 10**9]
        uhi = [-1, -1]
        for b_ in c["cols"]:
            for p in b_[1]:
                offs = []
                for axi, ilen, base in ((1, ih, 1), (2, iw, 2)):
                    ax = p.perm[axi]
                    ext = int(p.ext[ax])
                    if p.srcs:
                        slo = min(s[ax] for s in p.srcs) - int(p.lo[ax])
                        shi = max(s[ax] for s in p.srcs) - int(p.lo[ax])
                    else:
                        slo = shi = ext // 2
                    start = base + (ilen - 1) // 2 - (slo + shi) // 2
                    start = min(max(start, base), base + ilen - ext)
                    offs.append(start)
                    if p.srcs:
                        i01 = 0 if axi == 1 else 1
                        ulo[i01] = min(ulo[i01], start + slo)
                        uhi[i01] = max(uhi[i01], start + shi)
                p.offh, p.offw = offs
        if uhi[0] < 0:
            ulo, uhi = [1, 2], [ih, iw + 1]
        c["ext_k"] = []
        for k in range(1, N_ITERS + 1):
            h0 = max(1, ulo[0] - k)
            h1 = min(1 + ih, uhi[0] + k + 1)
            w0 = max(2, ulo[1] - k)
            w1 = min(2 + iw, uhi[1] + k + 1)
            w0 -= w0 % 2
            wn = w1 - w0
            wn += wn % 2
            c["ext_k"].append((int(h0), int(h1), int(w0), int(w0 + wn)))

    # max sources per (partition row, col) -> K one-hot passes per class
    for c in cfg.classes:
        c["K"] = 1
    cnt = {}
    for p in pieces:
        for s in p.srcs:
            key = (p.cls, p.core, p.col,
                   p.p0 + int(s[p.perm[0]] - p.lo[p.perm[0]]))
            cnt[key] = cnt.get(key, 0) + 1
    for c in cfg.classes:
        ks = [v for k, v in cnt.items() if k[0] == c["cls"]]
        c["K"] = max(ks) if ks else 1

    cfg.pieces = pieces
    return cfg


# --------------------------------------------------------------------------
# schedule: op list with engines + simulated times
# --------------------------------------------------------------------------

T_BARRIER = 7100.0   # engines free after framework preamble
T_DATA = 10400.0     # tgt/tm DMA visible
T_PR0 = 11000.0      # pr_0 visible
T_PR1 = 11900.0      # pr_1 visible
SEM = 110.0


def _c_dve(n):   return 165 + 0.52 * n
def _c_dve32(n): return 255 + 0.84 * n
def _c_pool(n):  return 160 + 1.75 * n
def _c_pool32(n): return 200 + 2.2 * n
def _c_pe(n):    return 190 + 0.80 * n
def _c_act(n):   return 256 + 0.84 * n


class SOp:
    """Schedule op: kind in {iota, onehot, w1, w2, h1, h2, mma, mmb, mmc,
    hop, mult, odma}; slices carried as metadata for the emitter."""
    __slots__ = ("kind", "ci", "it", "eng", "dur", "deps", "meta",
                 "t_start", "t_end", "idx")

    def __init__(self, kind, ci, it, eng, dur, deps, **meta):
        self.kind, self.ci, self.it, self.eng = kind, ci, it, eng
        self.dur = dur
        self.deps = [d for d in deps if d is not None]
        self.meta = meta
        self.t_start = self.t_end = None


def default_choices(cfg):
    """choices[(ci,it)] = dict(n_t, w1, w2, h1, h2, mult=[(mode,eng),...])
    mode in {direct, hop}; engines in {dve, pool}."""
    ch = {}
    for ci, c in enumerate(cfg.classes):
        for it in range(N_ITERS):
            ch[(ci, it)] = dict(n_t=1, w1="dve", w2="dve", h1="dve",
                                h2="dve", mult=[("hop", "dve")] * 4)
    return ch


def search_choices(cfg, iters=12000, seed=7):
    import random
    rnd = random.Random(seed)
    best = default_choices(cfg)
    ops = make_schedule(cfg, best)
    bmk = max(o.t_end for o in ops)
    keys = list(best.keys())
    for _ in range(iters):
        cand = {k: dict(v, mult=list(v["mult"])) for k, v in best.items()}
        for _ in range(rnd.choice((1, 1, 2))):
            k = rnd.choice(keys)
            v = cand[k]
            f = rnd.choice(("n_t", "w1", "w2", "h1", "h2", "mult"))
            if f == "n_t":
                v["n_t"] = rnd.choice((0, 1, 2))
            elif f == "mult":
                i = rnd.randrange(4)
                v["mult"][i] = (rnd.choice(("direct", "hop")),
                                rnd.choice(("dve", "pool")))
            else:
                v[f] = rnd.choice(("dve", "pool"))
        try:
            mk = max(o.t_end for o in make_schedule(cfg, cand))
        except Exception:
            continue
        if mk < bmk:
            best, bmk = cand, mk
    return best, bmk


def make_schedule(cfg, choices=None):
    """Deterministic scheduler given choices. Returns ops sorted by start."""
    if choices is None:
        choices = default_choices(cfg)
    ops = []
    eng_t = {"dve": T_BARRIER, "pool": T_BARRIER, "pe": T_BARRIER,
             "act": T_BARRIER, "sp": T_BARRIER + 2500}

    def place(op, earliest=0.0):
        t = max(eng_t[op.eng], earliest,
                *([d.t_end + (SEM if d.eng != op.eng else 0.0)
                   for d in op.deps] or [0.0]))
        op.t_start = t
        op.t_end = t + op.dur
        eng_t[op.eng] = op.t_end
        op.idx = len(ops)
        ops.append(op)
        return op

    # prologue: per-class iota (pool), one-hot (dve; + K-1 STT passes)
    onehot_done = {}
    iotas = {}
    for ci, c in enumerate(cfg.classes):
        n = c["bh"] * c["bw"]
        iotas[ci] = place(SOp("iota", ci, -1, "pool", 200 + 1.4 * n, []),
                          earliest=T_BARRIER)
    for ci, c in enumerate(cfg.classes):
        n = c["bh"] * c["bw"]
        last = iotas[ci]
        for jj in range(c["j"]):
            for k in range(c["K"]):
                last = place(SOp("onehot", ci, -1, "dve", 255 + 0.9 * n,
                                 [iotas[ci], last], jj=jj, k=k),
                             earliest=T_DATA)
        onehot_done[ci] = last

    umult = {(ci, -1): [onehot_done[ci]] for ci in range(len(cfg.classes))}
    tapres = {}
    mmres = {}

    def geom(ci, it):
        c = cfg.classes[ci]
        h0, h1, w0, w1 = c["ext_k"][it]
        return h0, h1, w0, w1, h1 - h0, w1 - w0

    def emit_taps(ci, it, n_t):
        ch = choices[(ci, it)]
        h0, h1, w0, w1, rows, wn = geom(ci, it)
        dep = umult[(ci, it - 1)]
        Nw = (rows + 2) * wn
        e = ch["w1"]
        w1op = place(SOp("w1", ci, it, e,
                         (_c_dve if e == "dve" else _c_pool)(Nw), dep))
        e = ch["w2"]
        w2op = place(SOp("w2", ci, it, e,
                         (_c_dve if e == "dve" else _c_pool)(Nw), [w1op]))
        h1op = h2op = None
        N = rows * wn
        if n_t >= 1:
            e = ch["h1"]
            h1op = place(SOp("h1", ci, it, e,
                             (_c_dve if e == "dve" else _c_pool)(N), [w2op]))
        if n_t == 2:
            e = ch["h2"]
            h2op = place(SOp("h2", ci, it, e,
                             (_c_dve if e == "dve" else _c_pool)(N), [h1op]))
        tapres[(ci, it)] = (w2op, h1op, h2op)
        return w2op

    def emit_mms(ci, it, n_t):
        c = cfg.classes[ci]
        h0, h1, w0, w1, rows, wn = geom(ci, it)
        w2op, h1op, h2op = tapres[(ci, it)]
        nch, rpc = _chunks(rows, wn)
        if ci == 1 and nch == 1 and rows >= 10:
            nch, rpc = 2, -(-rows // 2)
        res = []
        for jj in range(c["j"]):
            for ch in range(nch):
                r0 = h0 + ch * rpc
                nr = min(rpc, h1 - r0)
                if nr <= 0:
                    continue
                N = nr * wn
                if n_t == 2:
                    last = place(SOp("mm", ci, it, "pe", _c_pe(N), [h2op],
                                     jj=jj, r0=r0, nr=nr, src="t2",
                                     dh=0, start=True, stop=True))
                elif n_t == 1:
                    m1 = place(SOp("mm", ci, it, "pe", _c_pe(N), [w2op],
                                   jj=jj, r0=r0, nr=nr, src="t1", dh=0,
                                   start=True, stop=False))
                    last = place(SOp("mm", ci, it, "pe", _c_pe(N),
                                     [h1op, m1], jj=jj, r0=r0, nr=nr,
                                     src="t2", dh=0, start=False, stop=True))
                else:
                    m1 = place(SOp("mm", ci, it, "pe", _c_pe(N), [w2op],
                                   jj=jj, r0=r0, nr=nr, src="t1", dh=0,
                                   start=True, stop=False))
                    m2 = place(SOp("mm", ci, it, "pe", _c_pe(N), [w2op, m1],
                                   jj=jj, r0=r0, nr=nr, src="t1", dh=-1,
                                   start=False, stop=False))
                    last = place(SOp("mm", ci, it, "pe", _c_pe(N),
                                     [w2op, m2], jj=jj, r0=r0, nr=nr,
                                     src="t1", dh=1, start=False, stop=True))
                res.append((jj, r0, nr, N, last))
        mmres[(ci, it)] = res

    def emit_mults(ci, it):
        ch = choices[(ci, it)]
        pr_t = T_PR0 if ci == 0 else T_PR1
        outs = []
        for ki, (jj, r0, nr, N, mmop) in enumerate(mmres[(ci, it)]):
            mode, eng = ch["mult"][min(ki, len(ch["mult"]) - 1)]
            if mode == "direct":
                m = place(SOp("mult", ci, it, "dve", _c_dve32(N), [mmop],
                              jj=jj, r0=r0, nr=nr, src="psum"),
                          earliest=pr_t)
            else:
                hop = place(SOp("hop", ci, it, "act", _c_act(N), [mmop],
                                jj=jj, r0=r0, nr=nr))
                cost = _c_dve(N) if eng == "dve" else _c_pool(N)
                m = place(SOp("mult", ci, it, eng, cost, [hop],
                              jj=jj, r0=r0, nr=nr, src="t2"),
                          earliest=pr_t)
            outs.append(m)
        umult[(ci, it)] = outs
        return outs

    def pick_nt(ci, it):
        return choices[(ci, it)]["n_t"]

    nC = len(cfg.classes)
    if nC == 1:
        for it in range(N_ITERS):
            nt = pick_nt(0, it)
            emit_taps(0, it, nt)
            emit_mms(0, it, nt)
            emit_mults(0, it)
            if it >= 7:
                place(SOp("odma", 0, it, "sp", 650, umult[(0, it)]))
    else:
        pend = False
        for it in range(N_ITERS - 1):
            nt0 = pick_nt(0, it)
            emit_taps(0, it, nt0)
            emit_mms(0, it, nt0)
            if pend:
                emit_mults(1, it - 1)
                if it - 1 >= 7:
                    place(SOp("odma", 1, it - 1, "sp", 650,
                              umult[(1, it - 1)]))
            emit_mults(0, it)
            if it >= 7:
                place(SOp("odma", 0, it, "sp", 650, umult[(0, it)]))
            nt1 = pick_nt(1, it)
            emit_taps(1, it, nt1)
            emit_mms(1, it, nt1)
            pend = True
        it = N_ITERS - 1
        emit_mults(1, it - 1)
        place(SOp("odma", 1, it - 1, "sp", 650, umult[(1, it - 1)]))
        nt1 = pick_nt(1, it)
        emit_taps(1, it, nt1)
        emit_mms(1, it, nt1)
        nt0 = pick_nt(0, it)
        emit_taps(0, it, nt0)
        emit_mms(0, it, nt0)
        emit_mults(1, it)
        place(SOp("odma", 1, it, "sp", 650, umult[(1, it)]))
        emit_mults(0, it)
        place(SOp("odma", 0, it, "sp", 650, umult[(0, it)]))

    ops.sort(key=lambda o: (o.t_start, o.idx))
    return ops


# --------------------------------------------------------------------------
# host-side data packing
# --------------------------------------------------------------------------

def build_inputs(cfg, logits):
    bf16 = ml_dtypes.bfloat16
    in_maps = [dict() for _ in range(N_CORES)]
    for ci, c in enumerate(cfg.classes):
        j, bh, bw = c["j"], c["bh"], c["bw"]
        for core in range(N_CORES):
            in_maps[core][f"pr_{ci}"] = np.zeros((128, j, bh, bw), bf16)
            in_maps[core][f"tm_{ci}"] = np.zeros((128, j, 128), bf16)
            in_maps[core][f"tg_{ci}"] = np.full((128, j, c["K"]), -1.0,
                                                np.float32)

    inv27 = np.float32(1.0 / 27.0)
    for p in cfg.pieces:
        ci = next(i for i, c in enumerate(cfg.classes) if c["cls"] == p.cls)
        c = cfg.classes[ci]
        lo, hi, perm = p.lo, p.hi, p.perm
        pe = int(p.ext[perm[0]])
        eh = int(p.ext[perm[1]])
        ew = int(p.ext[perm[2]])
        sl = tuple(slice(int(lo[a]), int(hi[a]) + 1) for a in range(3))
        oh, ow = p.offh, p.offw
        dlt = (logits[p.batch, 1][sl].astype(np.float32)
               - logits[p.batch, 0][sl].astype(np.float32)).transpose(perm)
        prob = 1.0 / (1.0 + np.exp(-dlt))
        in_maps[p.core][f"pr_{ci}"][p.p0:p.p0 + pe, p.col,
                                    oh:oh + eh, ow:ow + ew] = \
            prob.astype(bf16)
        tg = in_maps[p.core][f"tg_{ci}"]
        used = {}
        for s in p.srcs:
            q = (s[perm[0]] - lo[perm[0]], s[perm[1]] - lo[perm[1]],
                 s[perm[2]] - lo[perm[2]])
            row = p.p0 + q[0]
            k = used.get(row, 0)
            used[row] = k + 1
            tg[row, p.col, k] = float((oh + q[1]) * c["bw"] + (ow + q[2]))
        tm = in_maps[p.core][f"tm_{ci}"]
        for i in range(pe):
            for d_ in (-1, 0, 1):
                if 0 <= i + d_ < pe:
                    tm[p.p0 + i, p.col, p.p0 + i + d_] = inv27
    return in_maps


# --------------------------------------------------------------------------
# device kernel
# --------------------------------------------------------------------------

def build_nc(cfg, sched):
    import concourse.bacc as bacc
    import concourse.tile as tile
    from concourse import mybir

    nc = bacc.Bacc("TRN2")
    dram = {}
    for ci, c in enumerate(cfg.classes):
        j, bh, bw = c["j"], c["bh"], c["bw"]
        dram[f"pr_{ci}"] = nc.dram_tensor(
            f"pr_{ci}", [128, j, bh, bw], mybir.dt.bfloat16,
            kind="ExternalInput")
        dram[f"tm_{ci}"] = nc.dram_tensor(
            f"tm_{ci}", [128, j, 128], mybir.dt.bfloat16,
            kind="ExternalInput")
        dram[f"tg_{ci}"] = nc.dram_tensor(
            f"tg_{ci}", [128, j, c["K"]], mybir.dt.float32,
            kind="ExternalInput")
        for oi in (8, 9, 10):
            h0, h1, w0, w1 = c["ext_k"][oi - 1]
            dram[f"o{oi}_{ci}"] = nc.dram_tensor(
                f"o{oi}_{ci}", [128, j, h1 - h0, bw],
                mybir.dt.bfloat16, kind="ExternalOutput")

    with tile.TileContext(nc) as tc:
        with tc.tile_pool(name="sb", bufs=1) as sb, \
             tc.tile_pool(name="ps", bufs=7, space="PSUM") as pp:
            tiles = []
            for ci, c in enumerate(cfg.classes):
                j, bh, bw = c["j"], c["bh"], c["bw"]
                u_t = sb.tile([128, j, bh, bw], mybir.dt.bfloat16,
                              tag=f"u{ci}")
                t1_t = sb.tile([128, j, bh, bw], mybir.dt.bfloat16,
                               tag=f"t1{ci}")
                t2_t = sb.tile([128, j, bh, bw], mybir.dt.bfloat16,
                               tag=f"t2{ci}")
                pr_t = sb.tile([128, j, bh, bw], mybir.dt.bfloat16,
                               tag=f"pr{ci}")
                tm_t = sb.tile([128, j, 128], mybir.dt.bfloat16,
                               tag=f"tm{ci}")
                io_t = sb.tile([128, bh, bw], mybir.dt.float32,
                               tag=f"io{ci}")
                tg_t = sb.tile([128, j, c["K"]], mybir.dt.float32,
                               tag=f"tg{ci}")
                tiles.append(dict(u=u_t, t1=t1_t, t2=t2_t, pr=pr_t,
                                  tm=tm_t, io=io_t, tg=tg_t))

            # input DMAs: tiny tg first (gates one-hot), then tm, then pr
            for ci in range(len(cfg.classes)):
                nc.sync.dma_start(out=tiles[ci]["tg"][:],
                                  in_=dram[f"tg_{ci}"][:])
            for ci in range(len(cfg.classes)):
                nc.sync.dma_start(out=tiles[ci]["tm"][:],
                                  in_=dram[f"tm_{ci}"][:])
            for ci in range(len(cfg.classes)):
                nc.scalar.dma_start(out=tiles[ci]["pr"][:],
                                    in_=dram[f"pr_{ci}"][:])

            def tw(ns):
                tc.tile_set_cur_wait(ns * 1e-6)

            psum_of = {}
            cur = 0.0
            for op in sched:
                cur = max(cur, op.t_start)
                tw(cur)
                ci = op.ci
                c = cfg.classes[ci] if ci >= 0 else None
                t = tiles[ci] if ci >= 0 else None
                if op.kind == "iota":
                    nc.gpsimd.iota(t["io"][:],
                                   pattern=[[1, c["bh"] * c["bw"]]],
                                   base=0, channel_multiplier=0,
                                   allow_small_or_imprecise_dtypes=True)
                elif op.kind == "onehot":
                    jj, k = op.meta["jj"], op.meta["k"]
                    if k == 0:
                        nc.vector.tensor_scalar(
                            out=t["u"][:, jj], in0=t["io"][:],
                            scalar1=t["tg"][:, jj, k:k + 1], scalar2=None,
                            op0=mybir.AluOpType.is_equal)
                    else:
                        nc.vector.scalar_tensor_tensor(
                            out=t["u"][:, jj], in0=t["io"][:],
                            scalar=t["tg"][:, jj, k:k + 1],
                            in1=t["u"][:, jj],
                            op0=mybir.AluOpType.is_equal,
                            op1=mybir.AluOpType.add)
                elif op.kind in ("w1", "w2", "h1", "h2"):
                    h0, h1, w0, w1 = c["ext_k"][op.it]
                    g0, g1 = h0 - 1, h1 + 1
                    eng = nc.vector if op.eng == "dve" else nc.gpsimd
                    if op.kind == "w1":
                        eng.tensor_add(t["t1"][:, :, g0:g1, w0:w1],
                                       t["u"][:, :, g0:g1, w0 - 1:w1 - 1],
                                       t["u"][:, :, g0:g1, w0 + 1:w1 + 1])
                    elif op.kind == "w2":
                        eng.tensor_add(t["t1"][:, :, g0:g1, w0:w1],
                                       t["t1"][:, :, g0:g1, w0:w1],
                                       t["u"][:, :, g0:g1, w0:w1])
                    elif op.kind == "h1":
                        eng.tensor_add(t["t2"][:, :, h0:h1, w0:w1],
                                       t["t1"][:, :, h0 - 1:h1 - 1, w0:w1],
                                       t["t1"][:, :, h0 + 1:h1 + 1, w0:w1])
                    else:
                        eng.tensor_add(t["t2"][:, :, h0:h1, w0:w1],
                                       t["t2"][:, :, h0:h1, w0:w1],
                                       t["t1"][:, :, h0:h1, w0:w1])
                elif op.kind == "mm":
                    h0, h1, w0, w1 = c["ext_k"][op.it]
                    jj, r0, nr = op.meta["jj"], op.meta["r0"], op.meta["nr"]
                    dh = op.meta["dh"]
                    src = t[op.meta["src"]]
                    if op.meta["start"]:
                        ps = pp.tile([128, nr, w1 - w0], mybir.dt.float32,
                                     tag="ps")
                        psum_of[(ci, op.it, jj, r0)] = ps
                    else:
                        ps = psum_of[(ci, op.it, jj, r0)]
                    nc.tensor.matmul(
                        ps[:], t["tm"][:, jj, :],
                        src[:, jj, r0 + dh:r0 + dh + nr, w0:w1],
                        start=op.meta["start"], stop=op.meta["stop"])
                elif op.kind == "hop":
                    h0, h1, w0, w1 = c["ext_k"][op.it]
                    jj, r0, nr = op.meta["jj"], op.meta["r0"], op.meta["nr"]
                    ps = psum_of[(ci, op.it, jj, r0)]
                    nc.scalar.activation(
                        t["t2"][:, jj, r0:r0 + nr, w0:w1], ps[:],
                        mybir.ActivationFunctionType.Copy)
                elif op.kind == "mult":
                    h0, h1, w0, w1 = c["ext_k"][op.it]
                    jj, r0, nr = op.meta["jj"], op.meta["r0"], op.meta["nr"]
                    if op.meta["src"] == "psum":
                        src = psum_of[(ci, op.it, jj, r0)][:]
                    else:
                        src = t["t2"][:, jj, r0:r0 + nr, w0:w1]
                    eng = nc.vector if op.eng == "dve" else nc.gpsimd
                    eng.tensor_mul(t["u"][:, jj, r0:r0 + nr, w0:w1],
                                   src, t["pr"][:, jj, r0:r0 + nr, w0:w1])
                elif op.kind == "odma":
                    h0, h1, w0, w1 = c["ext_k"][op.it]
                    nc.sync.dma_start(out=dram[f"o{op.it + 1}_{ci}"][:],
                                      in_=t["u"][:, :, h0:h1, :])
    nc.compile()
    return nc


# --------------------------------------------------------------------------
# host-side finalization (same as v1)
# --------------------------------------------------------------------------

def finalize(cfg, results, eb):
    cls_idx = {c["cls"]: i for i, c in enumerate(cfg.classes)}
    mus = np.zeros((B, 3), dtype=np.float64)
    for p in cfg.pieces:
        ci = cls_idx[p.cls]
        pe = int(p.ext[p.perm[0]])
        for oi in (8, 9, 10):
            m = results[p.core][f"o{oi}_{ci}"][p.p0:p.p0 + pe, p.col]
            mus[p.batch, oi - 8] = max(mus[p.batch, oi - 8],
                                       float(m.max()))

    per_batch = []
    for b in range(B):
        cscale = 1.0
        for it in range(3):
            if mus[b, it] > 0:
                cscale = mus[b, it] + SMOOTH * cscale
        scores = []
        for e in eb[b]:
            val = 0.0
            for p in cfg.pieces:
                if p.batch != b:
                    continue
                if np.all(p.clo <= e) and np.all(e <= p.chi):
                    ci = cls_idx[p.cls]
                    c = cfg.classes[ci]
                    h0, h1, _, _ = c["ext_k"][N_ITERS - 1]
                    q = (int(e[p.perm[0]] - p.lo[p.perm[0]]),
                         int(e[p.perm[1]] - p.lo[p.perm[1]]),
                         int(e[p.perm[2]] - p.lo[p.perm[2]]))
                    hs = p.offh + q[1] - h0
                    ws = p.offw + q[2]
                    if 0 <= hs < h1 - h0:
                        val = float(results[p.core][f"o10_{ci}"]
                                    [p.p0 + q[0], p.col, hs, ws])
                    break
            scores.append(val / cscale)
        per_batch.append(1.0 - np.float32(np.mean(np.asarray(scores,
                                                             np.float32))))
    return np.array(np.mean(np.asarray(per_batch, np.float32)),
                    dtype=np.float32)


# --------------------------------------------------------------------------
# entry point
# --------------------------------------------------------------------------

def kernel(logits, labels, endpoints_a, endpoints_b):
    global LAST_RESULTS
    logits = np.asarray(logits)
    ea = np.asarray(endpoints_a).astype(np.int64)
    eb = np.asarray(endpoints_b).astype(np.int64)

    cfg = pack(make_pieces(ea))
    choices, _ = search_choices(cfg)
    sched = make_schedule(cfg, choices)
    in_maps = build_inputs(cfg, logits)
    nc = build_nc(cfg, sched)

    from concourse.bass_utils import run_bass_kernel_spmd
    res = run_bass_kernel_spmd(nc, in_maps, core_ids=list(range(N_CORES)))
    LAST_RESULTS = res
    return finalize(cfg, res.results, eb)


if __name__ == "__main__":
    ins = {k: np.load(f"/tmp/in_{k}.npy")
           for k in ("logits", "labels", "endpoints_a", "endpoints_b")}
    out = kernel(**ins)
    print("kernel loss:", repr(out))


# revision 4
# speedup vs baseline: 1.0194x; 1.0158x over previous
"""CAPE connectivity loss on 8 Trainium2 NeuronCores.

Math (reference): fg_prob = softmax(logits, ch_axis)[:, 1] = sigmoid(l1 - l0);
per batch, heat diffuses from 32 source voxels for 10 iterations of
  h = avg_pool3d_3x3x3(h) * prob;  h /= (max(h) + 1e-5)
then scores = h[endpoints_b], loss = mean over batches of (1 - mean(scores)).

Key structure exploited:
 1. The 3x3x3 box filter dilates support by exactly 1 voxel per iteration, so
    after 10 iterations heat is identically zero outside L-inf radius-10 balls
    around the 32 sources. Compute only on per-cluster regions (bbox+10),
    merged until the expanded regions are pairwise disjoint - then zero-BC
    diffusion per region is exact.
 2. The per-iteration max-normalization commutes with the linear
    pool*prob step: iterate u_{k+1} = pool(u_k)*prob unnormalized, record
    mu_k = max(u_k); then h_k = u_k / c_k with c_k = mu_k + SMOOTH*c_{k-1}.
    Zero-BC values outside a piece's owned claim only *underestimate* the true
    field, and the argmax voxel lies inside some owned claim, so the max over
    all region boxes equals the true global max.
 3. Oversized regions are split along an axis with a +10-voxel halo per
    internal cut; each piece's owned claim stays exact for all 10 iterations.

Device layout: pieces are rotated (largest axis -> partitions), their source
bboxes centered, and packed into columns of [128, j, bh, bw] bf16 SBUF tiles
in two width classes, with zero guard bands around each box (2 cols left so
w-windows stay >= 2). prob = sigmoid(l1-l0) is computed on the host for the
packed boxes only and shipped directly. Per iteration on a band that tracks
the dilated support (even-aligned w-windows for DVE 2x mode), the separable
3x3x3 pool splits engine-adaptively: the D-axis 3-tap is a PE matmul against
a block-tridiagonal 1/27 matrix; the W-axis 3-tap always rides on PE as
w-shifted accumulating matmuls; the H-axis 3-tap runs as 0/1/2 DVE row-shift
adds with the remainder folded into extra h-shifted matmuls (3/6/9 matmuls
per chunk) - chosen per (class, iteration) by a cost model balancing DVE vs
PE load. The *prob multiply runs on DVE, reading PSUM directly for small
bands or via a ScalarE PSUM->SBUF copy (idle engine) for large ones. u is
updated in place; band slabs of u_8/u_9/u_10 are DMAed out and the host
computes the global maxima (scale recurrence c_10 = mu_10 + eps*mu_9 +
eps^2*mu_8 + O(eps^3)) and gathers endpoint_b values. A burst of dummy
matmuls at kernel start warms the PE HAM clock gate during the input DMA.
"""

import ml_dtypes
import numpy as np

B, C, D, H, W = 2, 2, 128, 256, 256
N_PAIRS = 32
N_ITERS = 10
SMOOTH = 1e-5
R = N_ITERS  # diffusion reach in voxels
VOL = (D, H, W)

AXIS_CAP = 34   # max free-axis extent of a piece (split with halo beyond)
P_CAP = 128     # max partition-axis extent
A_FREE = 21     # class-A pieces have both free extents <= A_FREE
N_CORES = 8
N_WARM = 64     # PE warmup matmuls (HAM un-throttle needs ~3.4us busy)
HOP_MIN = 256   # min band area for the ScalarE PSUM->SBUF hop

# Results of the last device run (for test harness introspection).
LAST_RESULTS = None


# --------------------------------------------------------------------------
# planning: clusters -> regions -> pieces
# --------------------------------------------------------------------------

class Piece:
    __slots__ = ("lo", "hi", "clo", "chi", "srcs", "batch",
                 "perm", "cls", "core", "col", "p0", "offh", "offw")

    def __init__(self, lo, hi, clo, chi, srcs, batch):
        self.lo = np.asarray(lo); self.hi = np.asarray(hi)
        self.clo = np.asarray(clo); self.chi = np.asarray(chi)
        self.srcs = srcs
        self.batch = batch

    @property
    def ext(self):
        return self.hi - self.lo + 1


def _merge_clusters(pts):
    """Merge clusters until expanded regions (bbox+R) are pairwise disjoint."""
    clusters = [[i] for i in range(len(pts))]

    def bbox(c):
        p = pts[c]
        return p.min(0), p.max(0)

    changed = True
    while changed:
        changed = False
        out = []
        while clusters:
            c = clusters.pop()
            lo_c, hi_c = bbox(c)
            for k, o in enumerate(clusters):
                lo_o, hi_o = bbox(o)
                if np.all(lo_c - hi_o <= 2 * R) and np.all(lo_o - hi_c <= 2 * R):
                    clusters[k] = o + c
                    changed = True
                    break
            else:
                out.append(c)
        clusters = out
    return clusters


def _split(piece):
    """Split a piece until partition extent <= P_CAP and free extents <=
    AXIS_CAP. Claims are halved; each internal cut adds R halo."""
    out, done = [piece], []
    while out:
        p = out.pop()
        ext = p.ext
        order = np.argsort(-ext, kind="stable")
        ax = None
        if ext[order[0]] > P_CAP:
            ax = order[0]
        elif ext[order[1]] > AXIS_CAP:
            ax = order[1]
        elif ext[order[2]] > AXIS_CAP:
            ax = order[2]
        if ax is None:
            done.append(p)
            continue
        mid = (p.clo[ax] + p.chi[ax]) // 2
        for a, b in ((p.clo[ax], mid), (mid + 1, p.chi[ax])):
            nclo, nchi = p.clo.copy(), p.chi.copy()
            nclo[ax], nchi[ax] = a, b
            nlo, nhi = p.lo.copy(), p.hi.copy()
            nlo[ax] = max(a - R, p.lo[ax])
            nhi[ax] = min(b + R, p.hi[ax])
            srcs = [s for s in p.srcs if nlo[ax] <= s[ax] <= nhi[ax]]
            out.append(Piece(nlo, nhi, nclo, nchi, srcs, p.batch))
    return done


def make_pieces(ea):
    pieces = []
    for b in range(ea.shape[0]):
        pts = np.unique(ea[b], axis=0)
        for cl in _merge_clusters(pts):
            p = pts[cl]
            lo = np.maximum(p.min(0) - R, 0)
            hi = np.minimum(p.max(0) + R, np.asarray(VOL) - 1)
            base = Piece(lo, hi, lo, hi, [tuple(x) for x in p], b)
            pieces.extend(_split(base))
    return pieces


# --------------------------------------------------------------------------
# packing: pieces -> (class, core, col, partition offset) + schedule
# --------------------------------------------------------------------------

class Cfg:
    pass


def _chunks(rows, wn):
    nch = max(1, -(-(rows * wn) // 512))
    rpc = -(-rows // nch)
    while rpc * wn > 512:
        nch += 1
        rpc = -(-rows // nch)
    return nch, rpc


def pack(pieces):
    """Assign each piece a rotation + (class, core, col, p0); build the
    per-(class, iteration) band slabs, chunking, and engine schedule."""
    for p in pieces:
        ext = p.ext
        p.perm = tuple(int(i) for i in np.argsort(-ext, kind="stable"))

    def free_ext(p):
        e = p.ext
        return e[p.perm[1]], e[p.perm[2]]

    cls_of = {}
    for p in pieces:
        fh, fw = free_ext(p)
        p.cls = 0 if (fh <= A_FREE and fw <= A_FREE) else 1
        cls_of.setdefault(p.cls, []).append(p)

    cfg = Cfg()
    cfg.classes = []
    for cls in (0, 1):
        plist = cls_of.get(cls, [])
        if not plist:
            continue
        ih = max(int(free_ext(p)[0]) for p in plist)
        iw = max(int(free_ext(p)[1]) for p in plist)
        bh = ih + 2
        bw = iw + 4 + ((iw + 4) % 2)  # 2 guard cols left, >=2 right, even
        # first-fit decreasing bin pack by partition extent
        plist.sort(key=lambda p: -int(p.ext[p.perm[0]]))
        bins = []  # (used, [pieces])
        for p in plist:
            pe = int(p.ext[p.perm[0]])
            for b_ in bins:
                if b_[0] + pe <= 128:
                    p.p0 = b_[0]
                    b_[0] += pe
                    b_[1].append(p)
                    break
            else:
                p.p0 = 0
                bins.append([pe, [p]])
        cfg.classes.append(dict(cls=cls, ih=ih, iw=iw, bh=bh, bw=bw,
                                cols=bins))

    for c in cfg.classes:
        for i, b_ in enumerate(c["cols"]):
            for p in b_[1]:
                p.core = i % N_CORES
                p.col = i // N_CORES
        c["j"] = (len(c["cols"]) + N_CORES - 1) // N_CORES

    # Center each piece's source bbox in its box interior (rows start at 1,
    # cols at 2 so shifted matmul reads never cross the box start), then
    # per-class per-iteration band slabs (h0,h1,w0,w1) with the w-window
    # even-aligned for DVE 2x mode.
    for c in cfg.classes:
        ih, iw = c["ih"], c["iw"]
        ulo = [10**9, 10**9]
        uhi = [-1, -1]
        for b_ in c["cols"]:
            for p in b_[1]:
                offs = []
                for axi, ilen, base in ((1, ih, 1), (2, iw, 2)):
                    ax = p.perm[axi]
                    ext = int(p.ext[ax])
                    if p.srcs:
                        slo = min(s[ax] for s in p.srcs) - int(p.lo[ax])
                        shi = max(s[ax] for s in p.srcs) - int(p.lo[ax])
                    else:
                        slo = shi = ext // 2
                    start = base + (ilen - 1) // 2 - (slo + shi) // 2
                    start = min(max(start, base), base + ilen - ext)
                    offs.append(start)
                    if p.srcs:
                        i01 = 0 if axi == 1 else 1
                        ulo[i01] = min(ulo[i01], start + slo)
                        uhi[i01] = max(uhi[i01], start + shi)
                p.offh, p.offw = offs
        if uhi[0] < 0:  # no sources in this class at all
            ulo, uhi = [1, 2], [ih, iw + 1]
        c["ext_k"] = []
        for k in range(1, N_ITERS + 1):
            h0 = max(1, ulo[0] - k)
            h1 = min(1 + ih, uhi[0] + k + 1)
            w0 = max(2, ulo[1] - k)
            w1 = min(2 + iw, uhi[1] + k + 1)
            w0 -= w0 % 2
            wn = w1 - w0
            wn += wn % 2
            c["ext_k"].append((int(h0), int(h1), int(w0), int(w0 + wn)))

    # Engine schedule. The PE is power-throttled to ~50% duty under
    # sustained load (~1.2 ns/col effective), so DVE 2x (0.53 ns/elem) is
    # the fastest engine per element: the W-axis 3-tap always runs as 2 DVE
    # shift-adds; per (iteration, class) the H-axis 3-tap runs as n_t in
    # {2, 1, 0} further DVE ops (2 = full H on DVE + 1 matmul; 1 = H pair
    # on DVE + 2 matmuls; 0 = H folded into 3 h-shifted matmuls), greedily
    # balancing cumulative DVE vs PE busy-time. The psum->u multiply hops
    # through a chunk-pipelined ScalarE copy (idle engine) for bands >=
    # HOP_MIN so the DVE multiply runs from SBUF at 2x.
    dve = pe = 0.0
    for it in range(N_ITERS):
        for c in cfg.classes:
            j = c["j"]
            h0, h1, w0, w1 = c["ext_k"][it]
            rows, wn = h1 - h0, w1 - w0
            A = rows * wn * j
            nch, rpc = _chunks(rows, wn)
            ach = rpc * wn
            hop = A >= HOP_MIN
            mcost = (nch * (170 + 0.53 * ach) if hop
                     else nch * (192 + 1.25 * ach))
            best = None
            for n in (2, 1, 0):
                d = (2 + n) * (170 + 0.53 * A) + mcost * j
                pcost = nch * (3 - n) * (ach * 1.2 + 160) * j
                tot = max(dve + d, pe + pcost)
                if best is None or tot < best[0]:
                    best = (tot, n, d, pcost)
            _, n, d, pcost = best
            dve += d
            pe += pcost
            c.setdefault("sched", []).append(
                dict(n_t=n, hop=hop, nch=nch, rpc=rpc))
    cfg.pieces = pieces
    return cfg


# --------------------------------------------------------------------------
# host-side data packing
# --------------------------------------------------------------------------

def build_inputs(cfg, logits, ea):
    """Build per-core input arrays. Returns in_maps (list of dicts)."""
    bf16 = ml_dtypes.bfloat16
    in_maps = [dict() for _ in range(N_CORES)]
    for ci, c in enumerate(cfg.classes):
        j, bh, bw = c["j"], c["bh"], c["bw"]
        for core in range(N_CORES):
            # prob defaults to 0 outside regions: no heat leaks through box
            # cells beyond a (clipped) region slab, matching the true zero BC.
            in_maps[core][f"pr_{ci}"] = np.zeros((128, j, bh, bw), bf16)
            in_maps[core][f"u0_{ci}"] = np.zeros((128, j, bh, bw), bf16)
            in_maps[core][f"tm_{ci}"] = np.zeros((128, j, 128), bf16)

    inv27 = np.float32(1.0 / 27.0)
    for p in cfg.pieces:
        ci = next(i for i, c in enumerate(cfg.classes) if c["cls"] == p.cls)
        lo, hi, perm = p.lo, p.hi, p.perm
        pe = int(p.ext[perm[0]])
        eh = int(p.ext[perm[1]])
        ew = int(p.ext[perm[2]])
        sl = tuple(slice(int(lo[a]), int(hi[a]) + 1) for a in range(3))
        oh, ow = p.offh, p.offw
        dlt = (logits[p.batch, 1][sl].astype(np.float32)
               - logits[p.batch, 0][sl].astype(np.float32)).transpose(perm)
        prob = 1.0 / (1.0 + np.exp(-dlt))
        in_maps[p.core][f"pr_{ci}"][p.p0:p.p0 + pe, p.col,
                                    oh:oh + eh, ow:ow + ew] = \
            prob.astype(bf16)
        u0 = in_maps[p.core][f"u0_{ci}"]
        for s in p.srcs:
            q = (s[perm[0]] - lo[perm[0]], s[perm[1]] - lo[perm[1]],
                 s[perm[2]] - lo[perm[2]])
            u0[p.p0 + q[0], p.col, oh + q[1], ow + q[2]] = 1.0
        tm = in_maps[p.core][f"tm_{ci}"]
        for i in range(pe):
            for d_ in (-1, 0, 1):
                if 0 <= i + d_ < pe:
                    tm[p.p0 + i, p.col, p.p0 + i + d_] = inv27
    return in_maps


# --------------------------------------------------------------------------
# device kernel
# --------------------------------------------------------------------------

def build_nc(cfg):
    import concourse.bacc as bacc
    import concourse.tile as tile
    from concourse import mybir

    nc = bacc.Bacc("TRN2")
    dram = {}
    for ci, c in enumerate(cfg.classes):
        j, bh, bw = c["j"], c["bh"], c["bw"]
        for nm in ("pr", "u0"):
            dram[f"{nm}_{ci}"] = nc.dram_tensor(
                f"{nm}_{ci}", [128, j, bh, bw], mybir.dt.bfloat16,
                kind="ExternalInput")
        dram[f"tm_{ci}"] = nc.dram_tensor(
            f"tm_{ci}", [128, j, 128], mybir.dt.bfloat16, kind="ExternalInput")
        # full-width row ranges: per-partition contiguous runs so the DMA
        # moves ~2KB packets instead of one tiny packet per row
        for oi in (8, 9, 10):
            h0, h1, w0, w1 = c["ext_k"][oi - 1]
            dram[f"o{oi}_{ci}"] = nc.dram_tensor(
                f"o{oi}_{ci}", [128, j, h1 - h0, bw],
                mybir.dt.bfloat16, kind="ExternalOutput")

    with tile.TileContext(nc) as tc:
        with tc.tile_pool(name="sb", bufs=1) as sb, \
             tc.tile_pool(name="ps", bufs=7, space="PSUM") as pp:
            tiles = []
            for ci, c in enumerate(cfg.classes):
                j, bh, bw = c["j"], c["bh"], c["bw"]
                u = sb.tile([128, j, bh, bw], mybir.dt.bfloat16, tag=f"u{ci}")
                t1 = sb.tile([128, j, bh, bw], mybir.dt.bfloat16,
                             tag=f"t1{ci}")
                t2 = sb.tile([128, j, bh, bw], mybir.dt.bfloat16,
                             tag=f"t2{ci}")
                pr = sb.tile([128, j, bh, bw], mybir.dt.bfloat16,
                             tag=f"pr{ci}")
                tm = sb.tile([128, j, 128], mybir.dt.bfloat16, tag=f"tm{ci}")
                tiles.append((u, t1, t2, pr, tm))
            # inputs spread over three trigger queues (parallel DMA
            # engines); u0_0 gates the first taps, so its transfer is
            # split across two queues to halve the per-packet serial cost
            u0t = tiles[0][0]
            nc.sync.dma_start(out=u0t[0:64], in_=dram["u0_0"][0:64])
            nc.gpsimd.dma_start(out=u0t[64:128], in_=dram["u0_0"][64:128])
            for ci, c in enumerate(cfg.classes):
                u, t1, t2, pr, tm = tiles[ci]
                qa = nc.sync if ci == 0 else nc.gpsimd
                if ci > 0:
                    qa.dma_start(out=u[:], in_=dram[f"u0_{ci}"][:])
                qa.dma_start(out=tm[:], in_=dram[f"tm_{ci}"][:])
                nc.scalar.dma_start(out=pr[:], in_=dram[f"pr_{ci}"][:])

            def emit_taps(ci, it):
                # W-axis 3-tap on DVE (2 shift-adds), written 1 row wider
                # than the band so later H-tap/matmul reads touch only
                # written cells (u's ring is zero from the u0 DMA - no
                # memsets needed). Then the H-axis 3-tap as n_t in {2,1,0}
                # more DVE ops (the rest rides on PE as shifted matmuls).
                c = cfg.classes[ci]
                u, t1, t2, pr, tm = tiles[ci]
                h0, h1, w0, w1 = c["ext_k"][it]
                n_t = c["sched"][it]["n_t"]
                g0, g1 = h0 - 1, h1 + 1
                nc.vector.tensor_add(t1[:, :, g0:g1, w0:w1],
                                     u[:, :, g0:g1, w0 - 1:w1 - 1],
                                     u[:, :, g0:g1, w0 + 1:w1 + 1])
                nc.vector.tensor_add(t1[:, :, g0:g1, w0:w1],
                                     t1[:, :, g0:g1, w0:w1],
                                     u[:, :, g0:g1, w0:w1])
                if n_t >= 1:
                    nc.vector.tensor_add(t2[:, :, h0:h1, w0:w1],
                                         t1[:, :, h0 - 1:h1 - 1, w0:w1],
                                         t1[:, :, h0 + 1:h1 + 1, w0:w1])
                if n_t == 2:
                    nc.vector.tensor_add(t2[:, :, h0:h1, w0:w1],
                                         t2[:, :, h0:h1, w0:w1],
                                         t1[:, :, h0:h1, w0:w1])

            def emit_mms(ci, it):
                # D-axis tridiagonal matmul per chunk; residual H-axis taps
                # ride along as h-shifted accumulating matmuls. Then each
                # psum chunk hops through ScalarE (PSUM->SBUF bf16, idle
                # engine, overwriting t2) for bands >= HOP_MIN.
                c = cfg.classes[ci]
                u, t1, t2, pr, tm = tiles[ci]
                h0, h1, w0, w1 = c["ext_k"][it]
                sch = c["sched"][it]
                n_t, nch, rpc = sch["n_t"], sch["nch"], sch["rpc"]
                if n_t == 2:       # t2 = full H 3-tap sum
                    srcs = [(t2, 0)]
                elif n_t == 1:     # t2 = t1(h-1)+t1(h+1), center = t1
                    srcs = [(t2, 0), (t1, 0)]
                else:              # full H 3-tap as shifted matmuls
                    srcs = [(t1, -1), (t1, 0), (t1, 1)]
                psums = []
                for jj in range(c["j"]):
                    for ch in range(nch):
                        r0 = h0 + ch * rpc
                        nr = min(rpc, h1 - r0)
                        ps = pp.tile([128, nr, w1 - w0],
                                     mybir.dt.float32, tag="ps")
                        for mi, (src, dh) in enumerate(srcs):
                            nc.tensor.matmul(
                                ps[:],
                                tm[:, jj, :],
                                src[:, jj, r0 + dh:r0 + dh + nr, w0:w1],
                                start=(mi == 0),
                                stop=(mi == len(srcs) - 1))
                        psums.append((jj, r0, nr, ps))
                if sch["hop"]:
                    # chunk 0 skips the hop (its multiply reads PSUM
                    # directly) - it sits first on the critical path and
                    # saving the ScalarE round-trip starts it ~0.6us sooner
                    for ki, (jj, r0, nr, ps) in enumerate(psums):
                        if ki == 0 and len(psums) >= 2:
                            continue
                        nc.scalar.activation(
                            t2[:, jj, r0:r0 + nr, w0:w1], ps[:],
                            mybir.ActivationFunctionType.Copy)
                return psums

            def emit_mults(ci, it, psums):
                # u = psum * prob (1/27 folded into tm), chunk-level so
                # early chunks run while later chunks are still matmuling.
                c = cfg.classes[ci]
                u, t1, t2, pr, tm = tiles[ci]
                h0, h1, w0, w1 = c["ext_k"][it]
                hop = c["sched"][it]["hop"]
                for ki, (jj, r0, nr, ps) in enumerate(psums):
                    direct = (not hop) or (ki == 0 and len(psums) >= 2)
                    src = ps[:] if direct else t2[:, jj, r0:r0 + nr, w0:w1]
                    nc.vector.tensor_mul(u[:, jj, r0:r0 + nr, w0:w1],
                                         src, pr[:, jj, r0:r0 + nr, w0:w1])
                if it >= 7:
                    # gpsimd queue: idle mid-kernel, so the slab transfer
                    # starts immediately and the WAR on u clears sooner
                    nc.gpsimd.dma_start(out=dram[f"o{it + 1}_{ci}"][:],
                                        in_=u[:, :, h0:h1, :])

            # Software-pipelined schedule: class 1's psum->u multiplies are
            # deferred into the next iteration so the in-order DVE queue
            # always has ready work (class 0's taps/multiply) while class
            # 1's matmul+copy chain drains. The Tile scheduler would undo
            # this (its cost model assumes an unthrottled PE and hoists the
            # stalling multiplies), so each phase is pinned with a
            # monotonically increasing bass_wait_until_ts pseudo-time.
            def tw(ns):
                tc.tile_set_cur_wait(ns * 1e-6)

            def dur(ci, it, what):
                c = cfg.classes[ci]
                h0, h1, w0, w1 = c["ext_k"][it]
                A = (h1 - h0) * (w1 - w0) * c["j"]
                sch = c["sched"][it]
                if what == "taps":
                    return (2 + sch["n_t"]) * (190 + 0.55 * A)
                if what == "mult":
                    per = (170 + 0.53 * A / sch["nch"]) if sch["hop"] \
                        else (192 + 1.25 * A / sch["nch"])
                    return sch["nch"] * per
                ach = A / sch["nch"]
                return sch["nch"] * (3 - sch["n_t"]) * (ach * 1.2 + 160)

            if len(cfg.classes) == 1:
                for it in range(N_ITERS):
                    emit_taps(0, it)
                    emit_mults(0, it, emit_mms(0, it))
            else:
                pend = None
                t = 3000.0
                for it in range(N_ITERS - 1):
                    tw(t)
                    emit_taps(0, it)
                    tb = t + dur(0, it, "taps")
                    tw(tb)
                    ps0 = emit_mms(0, it)
                    if pend is not None:
                        tw(tb + 200)
                        emit_mults(1, it - 1, pend)
                        tb += 200 + dur(1, it - 1, "mult")
                    tw(tb + 100)
                    emit_mults(0, it, ps0)
                    tb += 100 + dur(0, it, "mult")
                    tw(tb)
                    emit_taps(1, it)
                    tb += dur(1, it, "taps")
                    tw(tb)
                    pend = emit_mms(1, it)
                    t = tb + 400
                # Final iteration with roles swapped: class 1 (the long
                # matmul+copy chain) is issued first and class 0's short
                # chain drains the tail.
                it = N_ITERS - 1
                tw(t)
                emit_mults(1, it - 1, pend)
                t += dur(1, it - 1, "mult")
                tw(t)
                emit_taps(1, it)
                t += dur(1, it, "taps")
                tw(t)
                ps1 = emit_mms(1, it)
                tw(t + 200)
                emit_taps(0, it)
                t += 200 + dur(0, it, "taps")
                tw(t)
                ps0 = emit_mms(0, it)
                tw(t + 1400)
                emit_mults(1, it, ps1)
                tw(t + 1400 + dur(1, it, "mult"))
                emit_mults(0, it, ps0)
    nc.compile()
    return nc


# --------------------------------------------------------------------------
# host-side finalization
# --------------------------------------------------------------------------

def finalize(cfg, results, eb):
    """results: list of per-core dicts with o8/o9/o10 band slabs."""
    cls_idx = {c["cls"]: i for i, c in enumerate(cfg.classes)}

    # Global maxima of u_8/u_9/u_10 from the slabs; c_10 = mu_10 + eps*mu_9
    # + eps^2*mu_8 + O(eps^3) with eps = SMOOTH = 1e-5 (~1e-15 truncation).
    mus = np.zeros((B, 3), dtype=np.float64)
    for p in cfg.pieces:
        ci = cls_idx[p.cls]
        pe = int(p.ext[p.perm[0]])
        for oi in (8, 9, 10):
            m = results[p.core][f"o{oi}_{ci}"][p.p0:p.p0 + pe, p.col]
            mus[p.batch, oi - 8] = max(mus[p.batch, oi - 8],
                                       float(m.max()))

    per_batch = []
    for b in range(B):
        cscale = 1.0
        for it in range(3):
            if mus[b, it] > 0:
                cscale = mus[b, it] + SMOOTH * cscale
        scores = []
        for e in eb[b]:
            val = 0.0
            for p in cfg.pieces:
                if p.batch != b:
                    continue
                if np.all(p.clo <= e) and np.all(e <= p.chi):
                    ci = cls_idx[p.cls]
                    c = cfg.classes[ci]
                    h0, h1, _, _ = c["ext_k"][N_ITERS - 1]
                    q = (int(e[p.perm[0]] - p.lo[p.perm[0]]),
                         int(e[p.perm[1]] - p.lo[p.perm[1]]),
                         int(e[p.perm[2]] - p.lo[p.perm[2]]))
                    hs = p.offh + q[1] - h0
                    ws = p.offw + q[2]
                    if 0 <= hs < h1 - h0:
                        val = float(results[p.core][f"o10_{ci}"]
                                    [p.p0 + q[0], p.col, hs, ws])
                    break
            scores.append(val / cscale)
        per_batch.append(1.0 - np.float32(np.mean(np.asarray(scores,
                                                             np.float32))))
    return np.array(np.mean(np.asarray(per_batch, np.float32)),
                    dtype=np.float32)


# --------------------------------------------------------------------------
# entry point
# --------------------------------------------------------------------------

def kernel(logits, labels, endpoints_a, endpoints_b):
    global LAST_RESULTS
    logits = np.asarray(logits)
    ea = np.asarray(endpoints_a).astype(np.int64)
    eb = np.asarray(endpoints_b).astype(np.int64)

    cfg = pack(make_pieces(ea))
    in_maps = build_inputs(cfg, logits, ea)
    nc = build_nc(cfg)

    from concourse.bass_utils import run_bass_kernel_spmd
    res = run_bass_kernel_spmd(nc, in_maps, core_ids=list(range(N_CORES)))
    LAST_RESULTS = res
    return finalize(cfg, res.results, eb)


if __name__ == "__main__":
    ins = {k: np.load(f"/tmp/in_{k}.npy")
           for k in ("logits", "labels", "endpoints_a", "endpoints_b")}
    out = kernel(**ins)
    print("kernel loss:", repr(out))



# revision 5
# speedup vs baseline: 1.0645x; 1.0443x over previous
"""CAPE connectivity loss on 8 Trainium2 NeuronCores.

Math (reference): fg_prob = softmax(logits, ch_axis)[:, 1] = sigmoid(l1 - l0);
per batch, heat diffuses from 32 source voxels for 10 iterations of
  h = avg_pool3d_3x3x3(h) * prob;  h /= (max(h) + 1e-5)
then scores = h[endpoints_b], loss = mean over batches of (1 - mean(scores)).

Key structure exploited:
 1. The 3x3x3 box filter dilates support by exactly 1 voxel per iteration, so
    after 10 iterations heat is identically zero outside L-inf radius-10 balls
    around the 32 sources. Compute only on per-cluster regions (bbox+10),
    merged until the expanded regions are pairwise disjoint - then zero-BC
    diffusion per region is exact.
 2. The per-iteration max-normalization commutes with the linear
    pool*prob step: iterate u_{k+1} = pool(u_k)*prob unnormalized, record
    mu_k = max(u_k); then h_k = u_k / c_k with c_k = mu_k + SMOOTH*c_{k-1}.
    Zero-BC values outside a piece's owned claim only *underestimate* the true
    field, and the argmax voxel lies inside some owned claim, so the max over
    all region boxes equals the true global max.
 3. Oversized regions are split along an axis with a +10-voxel halo per
    internal cut; each piece's owned claim stays exact for all 10 iterations.

Device layout: pieces are rotated (largest axis -> partitions), their source
bboxes centered, and packed into columns of [128, j, bh, bw] bf16 SBUF tiles
in two width classes, with zero guard bands around each box (2 cols left so
w-windows stay >= 2). prob = sigmoid(l1-l0) is computed on the host for the
packed boxes only and shipped directly. Per iteration on a band that tracks
the dilated support (even-aligned w-windows for DVE 2x mode), the separable
3x3x3 pool splits engine-adaptively: the D-axis 3-tap is a PE matmul against
a block-tridiagonal 1/27 matrix; the W-axis 3-tap always rides on PE as
w-shifted accumulating matmuls; the H-axis 3-tap runs as 0/1/2 DVE row-shift
adds with the remainder folded into extra h-shifted matmuls (3/6/9 matmuls
per chunk) - chosen per (class, iteration) by a cost model balancing DVE vs
PE load. The *prob multiply runs on DVE, reading PSUM directly for small
bands or via a ScalarE PSUM->SBUF copy (idle engine) for large ones. u is
updated in place; band slabs of u_8/u_9/u_10 are DMAed out and the host
computes the global maxima (scale recurrence c_10 = mu_10 + eps*mu_9 +
eps^2*mu_8 + O(eps^3)) and gathers endpoint_b values. A burst of dummy
matmuls at kernel start warms the PE HAM clock gate during the input DMA.
"""

import ml_dtypes
import numpy as np

B, C, D, H, W = 2, 2, 128, 256, 256
N_PAIRS = 32
N_ITERS = 10
SMOOTH = 1e-5
R = N_ITERS  # diffusion reach in voxels
VOL = (D, H, W)

AXIS_CAP = 34   # max free-axis extent of a piece (split with halo beyond)
P_CAP = 128     # max partition-axis extent
A_FREE = 21     # class-A pieces have both free extents <= A_FREE
N_CORES = 8
N_WARM = 64     # PE warmup matmuls (HAM un-throttle needs ~3.4us busy)
HOP_MIN = 256   # min band area for the ScalarE PSUM->SBUF hop

# Results of the last device run (for test harness introspection).
LAST_RESULTS = None


# --------------------------------------------------------------------------
# planning: clusters -> regions -> pieces
# --------------------------------------------------------------------------

class Piece:
    __slots__ = ("lo", "hi", "clo", "chi", "srcs", "batch",
                 "perm", "cls", "core", "col", "p0", "offh", "offw")

    def __init__(self, lo, hi, clo, chi, srcs, batch):
        self.lo = np.asarray(lo); self.hi = np.asarray(hi)
        self.clo = np.asarray(clo); self.chi = np.asarray(chi)
        self.srcs = srcs
        self.batch = batch

    @property
    def ext(self):
        return self.hi - self.lo + 1


def _merge_clusters(pts):
    """Merge clusters until expanded regions (bbox+R) are pairwise disjoint."""
    clusters = [[i] for i in range(len(pts))]

    def bbox(c):
        p = pts[c]
        return p.min(0), p.max(0)

    changed = True
    while changed:
        changed = False
        out = []
        while clusters:
            c = clusters.pop()
            lo_c, hi_c = bbox(c)
            for k, o in enumerate(clusters):
                lo_o, hi_o = bbox(o)
                if np.all(lo_c - hi_o <= 2 * R) and np.all(lo_o - hi_c <= 2 * R):
                    clusters[k] = o + c
                    changed = True
                    break
            else:
                out.append(c)
        clusters = out
    return clusters


def _split(piece):
    """Split a piece until partition extent <= P_CAP and free extents <=
    AXIS_CAP. Claims are halved; each internal cut adds R halo."""
    out, done = [piece], []
    while out:
        p = out.pop()
        ext = p.ext
        order = np.argsort(-ext, kind="stable")
        ax = None
        if ext[order[0]] > P_CAP:
            ax = order[0]
        elif ext[order[1]] > AXIS_CAP:
            ax = order[1]
        elif ext[order[2]] > AXIS_CAP:
            ax = order[2]
        if ax is None:
            done.append(p)
            continue
        mid = (p.clo[ax] + p.chi[ax]) // 2
        for a, b in ((p.clo[ax], mid), (mid + 1, p.chi[ax])):
            nclo, nchi = p.clo.copy(), p.chi.copy()
            nclo[ax], nchi[ax] = a, b
            nlo, nhi = p.lo.copy(), p.hi.copy()
            nlo[ax] = max(a - R, p.lo[ax])
            nhi[ax] = min(b + R, p.hi[ax])
            srcs = [s for s in p.srcs if nlo[ax] <= s[ax] <= nhi[ax]]
            out.append(Piece(nlo, nhi, nclo, nchi, srcs, p.batch))
    return done


def make_pieces(ea):
    pieces = []
    for b in range(ea.shape[0]):
        pts = np.unique(ea[b], axis=0)
        for cl in _merge_clusters(pts):
            p = pts[cl]
            lo = np.maximum(p.min(0) - R, 0)
            hi = np.minimum(p.max(0) + R, np.asarray(VOL) - 1)
            base = Piece(lo, hi, lo, hi, [tuple(x) for x in p], b)
            pieces.extend(_split(base))
    return pieces


# --------------------------------------------------------------------------
# packing: pieces -> (class, core, col, partition offset) + schedule
# --------------------------------------------------------------------------

class Cfg:
    pass


def _chunks(rows, wn):
    nch = max(1, -(-(rows * wn) // 512))
    rpc = -(-rows // nch)
    while rpc * wn > 512:
        nch += 1
        rpc = -(-rows // nch)
    return nch, rpc


def pack(pieces):
    """Assign each piece a rotation + (class, core, col, p0); build the
    per-(class, iteration) band slabs, chunking, and engine schedule."""
    for p in pieces:
        ext = p.ext
        p.perm = tuple(int(i) for i in np.argsort(-ext, kind="stable"))

    def free_ext(p):
        e = p.ext
        return e[p.perm[1]], e[p.perm[2]]

    cls_of = {}
    for p in pieces:
        fh, fw = free_ext(p)
        p.cls = 0 if (fh <= A_FREE and fw <= A_FREE) else 1
        cls_of.setdefault(p.cls, []).append(p)

    cfg = Cfg()
    cfg.classes = []
    for cls in (0, 1):
        plist = cls_of.get(cls, [])
        if not plist:
            continue
        ih = max(int(free_ext(p)[0]) for p in plist)
        iw = max(int(free_ext(p)[1]) for p in plist)
        bh = ih + 2
        bw = iw + 4 + ((iw + 4) % 2)  # 2 guard cols left, >=2 right, even
        # first-fit decreasing bin pack by partition extent
        plist.sort(key=lambda p: -int(p.ext[p.perm[0]]))
        bins = []  # (used, [pieces])
        for p in plist:
            pe = int(p.ext[p.perm[0]])
            for b_ in bins:
                if b_[0] + pe <= 128:
                    p.p0 = b_[0]
                    b_[0] += pe
                    b_[1].append(p)
                    break
            else:
                p.p0 = 0
                bins.append([pe, [p]])
        cfg.classes.append(dict(cls=cls, ih=ih, iw=iw, bh=bh, bw=bw,
                                cols=bins))

    for c in cfg.classes:
        for i, b_ in enumerate(c["cols"]):
            for p in b_[1]:
                p.core = i % N_CORES
                p.col = i // N_CORES
        c["j"] = (len(c["cols"]) + N_CORES - 1) // N_CORES

    # Center each piece's source bbox in its box interior (rows start at 1,
    # cols at 2 so shifted matmul reads never cross the box start), then
    # per-class per-iteration band slabs (h0,h1,w0,w1) with the w-window
    # even-aligned for DVE 2x mode.
    for c in cfg.classes:
        ih, iw = c["ih"], c["iw"]
        ulo = [10**9, 10**9]
        uhi = [-1, -1]
        for b_ in c["cols"]:
            for p in b_[1]:
                offs = []
                for axi, ilen, base in ((1, ih, 1), (2, iw, 2)):
                    ax = p.perm[axi]
                    ext = int(p.ext[ax])
                    if p.srcs:
                        slo = min(s[ax] for s in p.srcs) - int(p.lo[ax])
                        shi = max(s[ax] for s in p.srcs) - int(p.lo[ax])
                    else:
                        slo = shi = ext // 2
                    start = base + (ilen - 1) // 2 - (slo + shi) // 2
                    start = min(max(start, base), base + ilen - ext)
                    offs.append(start)
                    if p.srcs:
                        i01 = 0 if axi == 1 else 1
                        ulo[i01] = min(ulo[i01], start + slo)
                        uhi[i01] = max(uhi[i01], start + shi)
                p.offh, p.offw = offs
        if uhi[0] < 0:  # no sources in this class at all
            ulo, uhi = [1, 2], [ih, iw + 1]
        c["ext_k"] = []
        for k in range(1, N_ITERS + 1):
            h0 = max(1, ulo[0] - k)
            h1 = min(1 + ih, uhi[0] + k + 1)
            w0 = max(2, ulo[1] - k)
            w1 = min(2 + iw, uhi[1] + k + 1)
            w0 -= w0 % 2
            wn = w1 - w0
            wn += wn % 2
            c["ext_k"].append((int(h0), int(h1), int(w0), int(w0 + wn)))

    # Engine schedule. The PE is power-throttled to ~50% duty under
    # sustained load (~1.2 ns/col effective), so DVE 2x (0.53 ns/elem) is
    # the fastest engine per element: the W-axis 3-tap always runs as 2 DVE
    # shift-adds; per (iteration, class) the H-axis 3-tap runs as n_t in
    # {2, 1, 0} further DVE ops (2 = full H on DVE + 1 matmul; 1 = H pair
    # on DVE + 2 matmuls; 0 = H folded into 3 h-shifted matmuls), greedily
    # balancing cumulative DVE vs PE busy-time. The psum->u multiply hops
    # through a chunk-pipelined ScalarE copy (idle engine) for bands >=
    # HOP_MIN so the DVE multiply runs from SBUF at 2x.
    dve = pe = 0.0
    for it in range(N_ITERS):
        for c in cfg.classes:
            j = c["j"]
            h0, h1, w0, w1 = c["ext_k"][it]
            rows, wn = h1 - h0, w1 - w0
            A = rows * wn * j
            nch, rpc = _chunks(rows, wn)
            ach = rpc * wn
            hop = A >= HOP_MIN
            mcost = (nch * (170 + 0.53 * ach) if hop
                     else nch * (192 + 1.25 * ach))
            best = None
            for n in (2, 1, 0):
                d = (2 + n) * (170 + 0.53 * A) + mcost * j
                pcost = nch * (3 - n) * (ach * 0.9 + 160) * j
                tot = max(dve + d, pe + pcost)
                if best is None or tot < best[0]:
                    best = (tot, n, d, pcost)
            _, n, d, pcost = best
            dve += d
            pe += pcost
            c.setdefault("sched", []).append(
                dict(n_t=n, hop=hop, nch=nch, rpc=rpc))
    cfg.pieces = pieces
    return cfg


# --------------------------------------------------------------------------
# host-side data packing
# --------------------------------------------------------------------------

def build_inputs(cfg, logits, ea):
    """Build per-core input arrays. Returns in_maps (list of dicts)."""
    bf16 = ml_dtypes.bfloat16
    in_maps = [dict() for _ in range(N_CORES)]
    for ci, c in enumerate(cfg.classes):
        j, bh, bw = c["j"], c["bh"], c["bw"]
        for core in range(N_CORES):
            # prob defaults to 0 outside regions: no heat leaks through box
            # cells beyond a (clipped) region slab, matching the true zero BC.
            in_maps[core][f"pr_{ci}"] = np.zeros((128, j, bh, bw), bf16)
            in_maps[core][f"u0_{ci}"] = np.zeros((128, j, bh, bw), bf16)
            in_maps[core][f"tm_{ci}"] = np.zeros((128, j, 128), bf16)

    inv27 = np.float32(1.0 / 27.0)
    for p in cfg.pieces:
        ci = next(i for i, c in enumerate(cfg.classes) if c["cls"] == p.cls)
        lo, hi, perm = p.lo, p.hi, p.perm
        pe = int(p.ext[perm[0]])
        eh = int(p.ext[perm[1]])
        ew = int(p.ext[perm[2]])
        sl = tuple(slice(int(lo[a]), int(hi[a]) + 1) for a in range(3))
        oh, ow = p.offh, p.offw
        dlt = (logits[p.batch, 1][sl].astype(np.float32)
               - logits[p.batch, 0][sl].astype(np.float32)).transpose(perm)
        prob = 1.0 / (1.0 + np.exp(-dlt))
        in_maps[p.core][f"pr_{ci}"][p.p0:p.p0 + pe, p.col,
                                    oh:oh + eh, ow:ow + ew] = \
            prob.astype(bf16)
        u0 = in_maps[p.core][f"u0_{ci}"]
        for s in p.srcs:
            q = (s[perm[0]] - lo[perm[0]], s[perm[1]] - lo[perm[1]],
                 s[perm[2]] - lo[perm[2]])
            u0[p.p0 + q[0], p.col, oh + q[1], ow + q[2]] = 1.0
        tm = in_maps[p.core][f"tm_{ci}"]
        for i in range(pe):
            for d_ in (-1, 0, 1):
                if 0 <= i + d_ < pe:
                    tm[p.p0 + i, p.col, p.p0 + i + d_] = inv27
    return in_maps


# --------------------------------------------------------------------------
# device kernel
# --------------------------------------------------------------------------

def build_nc(cfg):
    import concourse.bacc as bacc
    import concourse.tile as tile
    from concourse import mybir

    nc = bacc.Bacc("TRN2")
    dram = {}
    for ci, c in enumerate(cfg.classes):
        j, bh, bw = c["j"], c["bh"], c["bw"]
        for nm in ("pr", "u0"):
            dram[f"{nm}_{ci}"] = nc.dram_tensor(
                f"{nm}_{ci}", [128, j, bh, bw], mybir.dt.bfloat16,
                kind="ExternalInput")
        dram[f"tm_{ci}"] = nc.dram_tensor(
            f"tm_{ci}", [128, j, 128], mybir.dt.bfloat16, kind="ExternalInput")
        # full-width row ranges: per-partition contiguous runs so the DMA
        # moves ~2KB packets instead of one tiny packet per row
        for oi in (8, 9, 10):
            h0, h1, w0, w1 = c["ext_k"][oi - 1]
            dram[f"o{oi}_{ci}"] = nc.dram_tensor(
                f"o{oi}_{ci}", [128, j, h1 - h0, bw],
                mybir.dt.bfloat16, kind="ExternalOutput")

    with tile.TileContext(nc) as tc:
        with tc.tile_pool(name="sb", bufs=1) as sb, \
             tc.tile_pool(name="ps", bufs=7, space="PSUM") as pp:
            tiles = []
            for ci, c in enumerate(cfg.classes):
                j, bh, bw = c["j"], c["bh"], c["bw"]
                u = sb.tile([128, j, bh, bw], mybir.dt.bfloat16, tag=f"u{ci}")
                t1 = sb.tile([128, j, bh, bw], mybir.dt.bfloat16,
                             tag=f"t1{ci}")
                t2 = sb.tile([128, j, bh, bw], mybir.dt.bfloat16,
                             tag=f"t2{ci}")
                pr = sb.tile([128, j, bh, bw], mybir.dt.bfloat16,
                             tag=f"pr{ci}")
                tm = sb.tile([128, j, 128], mybir.dt.bfloat16, tag=f"tm{ci}")
                tiles.append((u, t1, t2, pr, tm))
            # inputs spread over three trigger queues (parallel DMA
            # engines); u0_0 gates the first taps, so its transfer is
            # split across two queues to halve the per-packet serial cost
            u0t = tiles[0][0]
            nc.sync.dma_start(out=u0t[0:64], in_=dram["u0_0"][0:64])
            nc.gpsimd.dma_start(out=u0t[64:128], in_=dram["u0_0"][64:128])
            for ci, c in enumerate(cfg.classes):
                u, t1, t2, pr, tm = tiles[ci]
                qa = nc.sync if ci == 0 else nc.gpsimd
                if ci > 0:
                    qa.dma_start(out=u[:], in_=dram[f"u0_{ci}"][:])
                qa.dma_start(out=tm[:], in_=dram[f"tm_{ci}"][:])
                nc.scalar.dma_start(out=pr[:], in_=dram[f"pr_{ci}"][:])

            def emit_taps(ci, it):
                # W-axis 3-tap on DVE (2 shift-adds), written 1 row wider
                # than the band so later H-tap/matmul reads touch only
                # written cells (u's ring is zero from the u0 DMA - no
                # memsets needed). Then the H-axis 3-tap as n_t in {2,1,0}
                # more DVE ops (the rest rides on PE as shifted matmuls).
                c = cfg.classes[ci]
                u, t1, t2, pr, tm = tiles[ci]
                h0, h1, w0, w1 = c["ext_k"][it]
                n_t = c["sched"][it]["n_t"]
                g0, g1 = h0 - 1, h1 + 1
                nc.vector.tensor_add(t1[:, :, g0:g1, w0:w1],
                                     u[:, :, g0:g1, w0 - 1:w1 - 1],
                                     u[:, :, g0:g1, w0 + 1:w1 + 1])
                nc.vector.tensor_add(t1[:, :, g0:g1, w0:w1],
                                     t1[:, :, g0:g1, w0:w1],
                                     u[:, :, g0:g1, w0:w1])
                if n_t >= 1:
                    nc.vector.tensor_add(t2[:, :, h0:h1, w0:w1],
                                         t1[:, :, h0 - 1:h1 - 1, w0:w1],
                                         t1[:, :, h0 + 1:h1 + 1, w0:w1])
                if n_t == 2:
                    nc.vector.tensor_add(t2[:, :, h0:h1, w0:w1],
                                         t2[:, :, h0:h1, w0:w1],
                                         t1[:, :, h0:h1, w0:w1])

            def emit_mms(ci, it):
                # D-axis tridiagonal matmul per chunk; residual H-axis taps
                # ride along as h-shifted accumulating matmuls. Then each
                # psum chunk hops through ScalarE (PSUM->SBUF bf16, idle
                # engine, overwriting t2) for bands >= HOP_MIN.
                c = cfg.classes[ci]
                u, t1, t2, pr, tm = tiles[ci]
                h0, h1, w0, w1 = c["ext_k"][it]
                sch = c["sched"][it]
                n_t, nch, rpc = sch["n_t"], sch["nch"], sch["rpc"]
                if n_t == 2:       # t2 = full H 3-tap sum
                    srcs = [(t2, 0)]
                elif n_t == 1:     # center (t1) first: it depends only on
                    # the W taps, so it overlaps H1 on DVE; t2 accumulates
                    srcs = [(t1, 0), (t2, 0)]
                else:              # full H 3-tap as shifted matmuls
                    srcs = [(t1, 0), (t1, -1), (t1, 1)]
                psums = []
                for jj in range(c["j"]):
                    for ch in range(nch):
                        r0 = h0 + ch * rpc
                        nr = min(rpc, h1 - r0)
                        ps = pp.tile([128, nr, w1 - w0],
                                     mybir.dt.float32, tag="ps")
                        for mi, (src, dh) in enumerate(srcs):
                            nc.tensor.matmul(
                                ps[:],
                                tm[:, jj, :],
                                src[:, jj, r0 + dh:r0 + dh + nr, w0:w1],
                                start=(mi == 0),
                                stop=(mi == len(srcs) - 1))
                        psums.append((jj, r0, nr, ps))
                if sch["hop"]:
                    # chunk 0 skips the hop (its multiply reads PSUM
                    # directly) - it sits first on the critical path and
                    # saving the ScalarE round-trip starts it ~0.6us sooner
                    for ki, (jj, r0, nr, ps) in enumerate(psums):
                        if ki == 0 and len(psums) >= 2:
                            continue
                        nc.scalar.activation(
                            t2[:, jj, r0:r0 + nr, w0:w1], ps[:],
                            mybir.ActivationFunctionType.Copy)
                return psums

            def emit_mults(ci, it, psums):
                # u = psum * prob (1/27 folded into tm), chunk-level so
                # early chunks run while later chunks are still matmuling.
                c = cfg.classes[ci]
                u, t1, t2, pr, tm = tiles[ci]
                h0, h1, w0, w1 = c["ext_k"][it]
                hop = c["sched"][it]["hop"]
                for ki, (jj, r0, nr, ps) in enumerate(psums):
                    direct = (not hop) or (ki == 0 and len(psums) >= 2)
                    src = ps[:] if direct else t2[:, jj, r0:r0 + nr, w0:w1]
                    nc.vector.tensor_mul(u[:, jj, r0:r0 + nr, w0:w1],
                                         src, pr[:, jj, r0:r0 + nr, w0:w1])
                if it >= 7:
                    # gpsimd queue: idle mid-kernel, so the slab transfer
                    # starts immediately and the WAR on u clears sooner
                    nc.gpsimd.dma_start(out=dram[f"o{it + 1}_{ci}"][:],
                                        in_=u[:, :, h0:h1, :])

            # Software-pipelined schedule: class 1's psum->u multiplies are
            # deferred into the next iteration so the in-order DVE queue
            # always has ready work (class 0's taps/multiply) while class
            # 1's matmul+copy chain drains. The Tile scheduler would undo
            # this (its cost model assumes an unthrottled PE and hoists the
            # stalling multiplies), so each phase is pinned with a
            # monotonically increasing bass_wait_until_ts pseudo-time.
            def tw(ns):
                tc.tile_set_cur_wait(ns * 1e-6)

            def dur(ci, it, what):
                c = cfg.classes[ci]
                h0, h1, w0, w1 = c["ext_k"][it]
                A = (h1 - h0) * (w1 - w0) * c["j"]
                sch = c["sched"][it]
                if what == "taps":
                    return (2 + sch["n_t"]) * (190 + 0.55 * A)
                if what == "mult":
                    per = (170 + 0.53 * A / sch["nch"]) if sch["hop"] \
                        else (192 + 1.25 * A / sch["nch"])
                    return sch["nch"] * per
                ach = A / sch["nch"]
                return sch["nch"] * (3 - sch["n_t"]) * (ach * 0.9 + 160)

            if len(cfg.classes) == 1:
                for it in range(N_ITERS):
                    emit_taps(0, it)
                    emit_mults(0, it, emit_mms(0, it))
            else:
                pend = None
                t = 3000.0
                for it in range(N_ITERS - 1):
                    tw(t)
                    emit_taps(0, it)
                    tb = t + dur(0, it, "taps")
                    tw(tb)
                    ps0 = emit_mms(0, it)
                    if pend is not None:
                        tw(tb + 200)
                        emit_mults(1, it - 1, pend)
                        tb += 200 + dur(1, it - 1, "mult")
                    tw(tb + 100)
                    emit_mults(0, it, ps0)
                    tb += 100 + dur(0, it, "mult")
                    tw(tb)
                    emit_taps(1, it)
                    tb += dur(1, it, "taps")
                    tw(tb)
                    pend = emit_mms(1, it)
                    t = tb + 400
                # Final iteration with roles swapped: class 1 (the long
                # matmul+copy chain) is issued first and class 0's short
                # chain drains the tail.
                it = N_ITERS - 1
                tw(t)
                emit_mults(1, it - 1, pend)
                t += dur(1, it - 1, "mult")
                tw(t)
                emit_taps(1, it)
                t += dur(1, it, "taps")
                tw(t)
                ps1 = emit_mms(1, it)
                tw(t + 200)
                emit_taps(0, it)
                t += 200 + dur(0, it, "taps")
                tw(t)
                ps0 = emit_mms(0, it)
                tw(t + 1400)
                emit_mults(1, it, ps1)
                tw(t + 1400 + dur(1, it, "mult"))
                emit_mults(0, it, ps0)
    nc.compile()
    return nc


# --------------------------------------------------------------------------
# host-side finalization
# --------------------------------------------------------------------------

def finalize(cfg, results, eb):
    """results: list of per-core dicts with o8/o9/o10 band slabs."""
    cls_idx = {c["cls"]: i for i, c in enumerate(cfg.classes)}

    # Global maxima of u_8/u_9/u_10 from the slabs; c_10 = mu_10 + eps*mu_9
    # + eps^2*mu_8 + O(eps^3) with eps = SMOOTH = 1e-5 (~1e-15 truncation).
    mus = np.zeros((B, 3), dtype=np.float64)
    for p in cfg.pieces:
        ci = cls_idx[p.cls]
        pe = int(p.ext[p.perm[0]])
        for oi in (8, 9, 10):
            m = results[p.core][f"o{oi}_{ci}"][p.p0:p.p0 + pe, p.col]
            mus[p.batch, oi - 8] = max(mus[p.batch, oi - 8],
                                       float(m.max()))

    per_batch = []
    for b in range(B):
        cscale = 1.0
        for it in range(3):
            if mus[b, it] > 0:
                cscale = mus[b, it] + SMOOTH * cscale
        scores = []
        for e in eb[b]:
            val = 0.0
            for p in cfg.pieces:
                if p.batch != b:
                    continue
                if np.all(p.clo <= e) and np.all(e <= p.chi):
                    ci = cls_idx[p.cls]
                    c = cfg.classes[ci]
                    h0, h1, _, _ = c["ext_k"][N_ITERS - 1]
                    q = (int(e[p.perm[0]] - p.lo[p.perm[0]]),
                         int(e[p.perm[1]] - p.lo[p.perm[1]]),
                         int(e[p.perm[2]] - p.lo[p.perm[2]]))
                    hs = p.offh + q[1] - h0
                    ws = p.offw + q[2]
                    if 0 <= hs < h1 - h0:
                        val = float(results[p.core][f"o10_{ci}"]
                                    [p.p0 + q[0], p.col, hs, ws])
                    break
            scores.append(val / cscale)
        per_batch.append(1.0 - np.float32(np.mean(np.asarray(scores,
                                                             np.float32))))
    return np.array(np.mean(np.asarray(per_batch, np.float32)),
                    dtype=np.float32)


# --------------------------------------------------------------------------
# entry point
# --------------------------------------------------------------------------

def kernel(logits, labels, endpoints_a, endpoints_b):
    global LAST_RESULTS
    logits = np.asarray(logits)
    ea = np.asarray(endpoints_a).astype(np.int64)
    eb = np.asarray(endpoints_b).astype(np.int64)

    cfg = pack(make_pieces(ea))
    in_maps = build_inputs(cfg, logits, ea)
    nc = build_nc(cfg)

    from concourse.bass_utils import run_bass_kernel_spmd
    res = run_bass_kernel_spmd(nc, in_maps, core_ids=list(range(N_CORES)))
    LAST_RESULTS = res
    return finalize(cfg, res.results, eb)


if __name__ == "__main__":
    ins = {k: np.load(f"/tmp/in_{k}.npy")
           for k in ("logits", "labels", "endpoints_a", "endpoints_b")}
    out = kernel(**ins)
    print("kernel loss:", repr(out))

